# revision 1
# baseline (speedup 1.0000x reference)
"""Trainium2 Bass kernel for nn_AdvancedCardiomyocyteGNN (GAT/GCN message passing).

Strategy (8 NeuronCores, SPMD single NEFF):
  - Nodes sharded across cores (1250 -> padded 1280 per core).
  - Node-wise GEMMs computed on the owning core; per-edge alpha projections
    (h . a_src / h . a_dst) are folded into the main GEMM weights on the host.
  - AllGather replicates the transformed node features (bf16) to all cores.
  - Edges partitioned by dst, sorted, grouped per 128-dst block; source rows
    are fetched with dma_gather (128 edges per chunk land on 128 partitions);
    segment softmax + weighted segment sum are computed as one-hot matmuls
    (S^T @ M accumulated in PSUM per dst block).
  - Graph-structure-dependent one-hot/scatter matrices and index tables are
    precomputed on the host (pure preprocessing of the integer edge list).
"""

import sys
import time

sys.path.insert(0, "/opt/trn_rl_repo")

import numpy as np
import ml_dtypes

import concourse.bass as bass
import concourse.tile as tile
from concourse import bacc, mybir
from concourse.bass_utils import run_bass_kernel_spmd

F32 = mybir.dt.float32
BF16 = mybir.dt.bfloat16
F8 = mybir.dt.float8e4
I16 = mybir.dt.int16
NPBF16 = ml_dtypes.bfloat16

NCORES = 8


def _rup(x, m):
    return (x + m - 1) // m * m


# ----------------------------------------------------------------------------
# Host-side graph preprocessing
# ----------------------------------------------------------------------------

def prep_graph(edge_index, n_nodes, n_loc, n_loc_pad, heads_dummy=None):
    """Partition edges (with self loops) by dst across cores, sort by dst,
    group per 128-dst block, pad each block to a per-block common chunk count.

    Returns dict with per-core index/scatter data and layout constants."""
    src = np.asarray(edge_index[0], dtype=np.int64)
    dst = np.asarray(edge_index[1], dtype=np.int64)
    loop = np.arange(n_nodes, dtype=np.int64)
    src = np.concatenate([src, loop])
    dst = np.concatenate([dst, loop])

    # gcn normalization (reference: deg over dst including self loops)
    deg = np.bincount(dst, minlength=n_nodes).astype(np.float64)
    dinv = np.where(deg > 0, deg ** -0.5, 0.0)
    ce_all = (dinv[src] * dinv[dst]).astype(np.float32)

    # padded node ids
    def pad_id(n):
        return (n // n_loc) * n_loc_pad + (n % n_loc)

    srcp = pad_id(src)
    dstp = pad_id(dst)

    core_of = dst // n_loc
    nblk = n_loc_pad // 128

    # per (core, blk) edge lists
    per_core = []
    for c in range(NCORES):
        m = core_of == c
        s, d, ce = srcp[m], dstp[m], ce_all[m]
        dloc = d - c * n_loc_pad
        order = np.argsort(dloc, kind="stable")
        s, dloc, ce = s[order], dloc[order], ce[order]
        blk = dloc // 128
        per_core.append((s, dloc, ce, blk))

    # per-block chunk count, common across cores
    K = np.zeros(nblk, dtype=np.int64)
    for c in range(NCORES):
        _, _, _, blk = per_core[c]
        cnt = np.bincount(blk, minlength=nblk)
        K = np.maximum(K, (cnt + 127) // 128)
    K = np.maximum(K, 1).astype(int)
    totch = int(K.sum())
    nidx = totch * 128

    idx16 = np.zeros((NCORES, 128, nidx // 16), dtype=np.int16)
    idx16d = np.zeros((NCORES, 128, nidx // 16), dtype=np.int16)
    s01 = np.zeros((NCORES, 128, totch * 128), dtype=NPBF16)
    sg = np.zeros((NCORES, 128, totch * 128), dtype=NPBF16)

    chunk_off = np.concatenate([[0], np.cumsum(K)])  # chunk offset per block

    for c in range(NCORES):
        s, dloc, ce, blk = per_core[c]
        idx_flat = np.zeros(nidx, dtype=np.int16)
        idxd_flat = np.zeros(nidx, dtype=np.int16)
        for b in range(nblk):
            m = blk == b
            sb_, db_, cb_ = s[m], dloc[m] - b * 128, ce[m]
            ne = len(sb_)
            base = chunk_off[b] * 128  # edge slot offset
            idx_flat[base : base + ne] = sb_.astype(np.int16)
            idxd_flat[base : base + ne] = (
                c * n_loc_pad + b * 128 + db_
            ).astype(np.int16)
            ch = base // 128 + np.arange(ne) // 128  # global chunk id
            e_in = np.arange(ne) % 128
            s01[c, e_in, ch * 128 + db_] = NPBF16(1.0)
            sg[c, e_in, ch * 128 + db_] = cb_.astype(NPBF16)
        idx16[c] = np.tile(idx_flat.reshape(-1, 16).T, (8, 1))
        idx16d[c] = np.tile(idxd_flat.reshape(-1, 16).T, (8, 1))

    return {
        "K": K,
        "totch": totch,
        "nidx": nidx,
        "chunk_off": chunk_off,
        "idx16": idx16,
        "idx16d": idx16d,
        "s01": s01,
        "sg": sg,
    }


def prep_weights(ip, k_pad):
    """Fold/concat/cast weights on the host. Returns dict of shared arrays."""
    f32 = np.float32
    w_gat1 = np.asarray(ip["W_gat1"], f32)  # [F_IN, 768]
    a_src1 = np.asarray(ip["a_src1"], f32)  # [6, 128]
    a_dst1 = np.asarray(ip["a_dst1"], f32)
    w_skip = np.asarray(ip["W_skip"], f32)  # [F_IN, 64]
    f_in = w_gat1.shape[0]
    h1 = a_src1.shape[0]
    c1 = a_src1.shape[1]
    ws1 = np.einsum("khc,hc->kh", w_gat1.reshape(f_in, h1, c1), a_src1)
    wd1 = np.einsum("khc,hc->kh", w_gat1.reshape(f_in, h1, c1), a_dst1)
    w1 = np.concatenate([w_gat1, w_skip, ws1, wd1], axis=1)  # [F_IN, 844]
    w1c = _rup(w1.shape[1], 16)
    w1p = np.zeros((k_pad, w1c), NPBF16)
    w1p[:f_in, : w1.shape[1]] = w1.astype(NPBF16)

    w_gat2 = np.asarray(ip["W_gat2"], f32)  # [128, 512]
    a_src2 = np.asarray(ip["a_src2"], f32)  # [4, 128]
    a_dst2 = np.asarray(ip["a_dst2"], f32)
    h2 = a_src2.shape[0]
    ws2 = np.einsum("khc,hc->kh", w_gat2.reshape(128, h2, c1), a_src2)
    wd2 = np.einsum("khc,hc->kh", w_gat2.reshape(128, h2, c1), a_dst2)
    w2 = np.concatenate([w_gat2, ws2, wd2], axis=1)  # [128, 520]
    w2p = w2.astype(NPBF16)

    def rep(v, cols=None):
        v = np.asarray(v, f32).reshape(-1)
        if cols is not None:
            vv = np.zeros(cols, f32)
            vv[: len(v)] = v
            v = vv
        return np.tile(v[None, :], (128, 1)).astype(f32)

    cblob = np.concatenate(
        [
            rep(ip["b_gat1"]),   # 0:128
            rep(ip["g1"]),       # 128:256
            rep(ip["be1"]),      # 256:384
            rep(ip["b_gcn1"]),   # 384:512
            rep(ip["b_gat2"]),   # 512:640
            rep(ip["g2"]),       # 640:768
            rep(ip["be2"]),      # 768:896
            rep(ip["b_gcn2"], 64),   # 896:960
            rep(ip["b_skip"], 64),   # 960:1024
            rep(ip["b_fuse"], 64),   # 1024:1088
            rep(ip["g3"], 64),       # 1088:1152
            rep(ip["be3"], 64),      # 1152:1216
            rep(ip["b_c1"], 32),     # 1216:1248
            rep(ip["b_c2"], 16),     # 1248:1264
            rep(ip["b_c3"], 8),      # 1264:1272
        ],
        axis=1,
    )

    wf = np.asarray(ip["W_fuse"], f32)  # [192, 64]
    wc1 = np.asarray(ip["W_c1"], f32)  # [64, 32]
    wc2 = np.asarray(ip["W_c2"], f32)  # [32, 16]
    wc3 = np.asarray(ip["W_c3"], f32)  # [16, 5]
    wc3p = np.zeros((wc3.shape[0], 8), np.float32)
    wc3p[:, : wc3.shape[1]] = wc3

    cbT = np.zeros((128, 4), f32)
    cbT[:32, 0] = np.asarray(ip["b_c1"], f32)
    cbT[:16, 1] = np.asarray(ip["b_c2"], f32)
    cbT[:5, 2] = np.asarray(ip["b_c3"], f32)

    return {
        "W1": w1p,
        "W2": w2p,
        "Wgcn1": np.asarray(ip["W_gcn1"], f32).astype(NPBF16),
        "Wgcn2": np.asarray(ip["W_gcn2"], f32).astype(NPBF16),
        "Wfuse": wf.astype(NPBF16),
        "Wc1": wc1.astype(NPBF16),
        "Wc2": wc2.astype(NPBF16),
        "Wc3": wc3p.astype(NPBF16),
        "cblob": cblob,
        "cbT": cbT,
        "ident": np.eye(128, dtype=NPBF16),
        "w1c": w1c,
    }


# ----------------------------------------------------------------------------
# Bass program builder
# ----------------------------------------------------------------------------

def build_nc(cfg):
    """cfg: dict with n_loc_pad, k_pad (F_IN padded), K (list per block),
    totch, nidx, w1c, h1=6, h2=4."""
    n_loc_pad = cfg["n_loc_pad"]
    k_pad = cfg["k_pad"]
    Kb = cfg["K"]
    totch = cfg["totch"]
    nidx = cfg["nidx"]
    w1c = cfg["w1c"]
    chunk_off = cfg["chunk_off"]
    nblk = n_loc_pad // 128
    ntile = nblk
    kch = k_pad // 128
    NP = NCORES * n_loc_pad
    H1, H2 = 6, 4
    FP8 = bool(cfg.get("fp8", False))
    GSZ = int(cfg.get("gsz", 6))
    MIXED = bool(cfg.get("mixed", False))
    GB = int(cfg.get("gbufs", 2))
    S01SYNC = bool(cfg.get("s01sync", False))
    POB = int(cfg.get("pobufs", 2))
    HD = F8 if FP8 else BF16
    if FP8:
        # fp8 rows: [h fp8 | alpha hi/lo as raw bf16 bytes | pad to 256B]
        ROW1 = _rup(H1 * 128 + 2 * 24, 256)   # 1024
        ROW2 = _rup(H2 * 128 + 2 * 16, 256)   # 768
    else:
        ROW1 = _rup(H1 * 128 + 24, 128)   # h(768)+asrc hi/lo+adst hi/lo
        ROW2 = _rup(H2 * 128 + 16, 128)
    C1 = H1 * 128
    C2 = H2 * 128
    AS1 = C1 + 24   # alpha block end (asrc hi/lo + adst hi/lo)
    AS2 = C2 + 16
    EPS = 1e-5

    stage_cap = cfg.get("stage_cap", 99)
    agg_cap = cfg.get("agg_cap", 99)
    repeat = cfg.get("repeat", 1)
    STAGE_MARKS.clear()
    nc = bacc.Bacc("TRN2", target_bir_lowering=False, debug=False,
                   num_devices=NCORES)

    def din(name, shape, dt):
        return nc.dram_tensor(name, shape, dt, kind="ExternalInput").ap()

    xT = din("xT", [k_pad, n_loc_pad], BF16)
    W1 = din("W1", [k_pad, w1c], BF16)
    W2 = din("W2", [128, 520], BF16)
    Wgcn1 = din("Wgcn1", [128, 128], BF16)
    Wgcn2 = din("Wgcn2", [128, 64], BF16)
    Wfuse = din("Wfuse", [192, 64], BF16)
    Wc1 = din("Wc1", [64, 32], BF16)
    Wc2 = din("Wc2", [32, 16], BF16)
    Wc3 = din("Wc3", [16, 8], BF16)
    cblob = din("cblob", [128, 1272], F32)
    ident = din("ident", [128, 128], BF16)
    idx16 = din("idx16", [128, nidx // 16], I16)
    idx16d = din("idx16d", [128, nidx // 16], I16)
    s01b = din("s01", [128, totch * 128], BF16)
    sgb = din("sg", [128, totch * 128], BF16)

    cbT = din("cbT", [128, 4], F32)
    out = nc.dram_tensor("out", [8, n_loc_pad], F32, kind="ExternalOutput").ap()

    rg = [list(range(NCORES))]

    with tile.TileContext(nc) as tc:
        with (
            tc.tile_pool(name="const", bufs=1) as cpool,
            tc.tile_pool(name="persist", bufs=1) as pp,
            tc.tile_pool(name="dram", bufs=1, space="DRAM") as dpool,
        ):
            # ---- constants / persistent tiles ----
            cb = cpool.tile([128, 1272], F32)
            nc.sync.dma_start(cb[:], cblob[:])
            idt = cpool.tile([128, 128], BF16)
            nc.sync.dma_start(idt[:], ident[:])
            idxs = cpool.tile([128, nidx // 16], I16)
            nc.sync.dma_start(idxs[:], idx16[:])
            idxsd = cpool.tile([128, nidx // 16], I16)
            nc.sync.dma_start(idxsd[:], idx16d[:])
            wgcn1_sb = cpool.tile([128, 128], BF16)
            nc.sync.dma_start(wgcn1_sb[:], Wgcn1[:])
            wgcn2_sb = cpool.tile([128, 64], BF16)
            nc.sync.dma_start(wgcn2_sb[:], Wgcn2[:])
            w2_sb = cpool.tile([128, 520], BF16)
            nc.sync.dma_start(w2_sb[:], W2[:])
            wf_sb = cpool.tile([128, 2, 64], BF16)
            nc.sync.dma_start(wf_sb[:, 0, :], Wfuse[0:128, :])
            nc.sync.dma_start(wf_sb[:64, 1, :], Wfuse[128:192, :])
            wc1_sb = cpool.tile([64, 32], BF16)
            nc.sync.dma_start(wc1_sb[:], Wc1[:])
            wc2_sb = cpool.tile([32, 16], BF16)
            nc.sync.dma_start(wc2_sb[:], Wc2[:])
            wc3_sb = cpool.tile([16, 8], BF16)
            nc.sync.dma_start(wc3_sb[:], Wc3[:])
            epsb = cpool.tile([128, 1], F32)
            nc.vector.memset(epsb[:], EPS)
            cbT_sb = cpool.tile([128, 4], F32)
            nc.sync.dma_start(cbT_sb[:], cbT[:])

            # persistent across stages
            skip_sb = pp.tile([128, ntile, 64], F32)
            x1gcn_T = pp.tile([128, ntile, 128], BF16)
            x2gcn_T = pp.tile([128, ntile, 128], BF16)  # only [:64] used

            # =============== Stage A: GEMM1 (x @ [Wgat1|Wskip|Ws|Wd]) =======
            for _rep in range(repeat):
              _mark(nc, f"A:gemm1 r{_rep}")
              # DRAM staging + shared AG outputs (per-rep: Shared tensors
              # must have a single collective writer)
              h1_stage = dpool.tile([n_loc_pad, ROW1], HD, tag=f"h1s{_rep}")
              H1full = dpool.tile([NP, ROW1], HD, addr_space="Shared",
                                  tag=f"H1f{_rep}")
              y1_stage = dpool.tile([n_loc_pad, 128], BF16, tag=f"y1s{_rep}")
              Y1full = dpool.tile([NP, 128], BF16, addr_space="Shared",
                                  tag=f"Y1f{_rep}")
              h2_stage = dpool.tile([n_loc_pad, ROW2], HD, tag=f"h2s{_rep}")
              H2full = dpool.tile([NP, ROW2], HD, addr_space="Shared",
                                  tag=f"H2f{_rep}")
              y2_stage = dpool.tile([n_loc_pad, 128], BF16, tag=f"y2s{_rep}")
              Y2full = dpool.tile([NP, 128], BF16, addr_space="Shared",
                                  tag=f"Y2f{_rep}")
              with (
                  tc.tile_pool(name="ax", bufs=1) as axp,
                  tc.tile_pool(name="aw", bufs=1) as awp,
                  tc.tile_pool(name="apsum", bufs=2, space="PSUM") as apsum,
                  tc.tile_pool(name="astage", bufs=3) as astage,
              ):
                  xT_sb = axp.tile([128, kch, n_loc_pad], BF16)
                  w1_sb = awp.tile([128, kch, w1c], BF16)
                  xT_r = xT.rearrange("(c p) n -> p c n", p=128)
                  W1_r = W1.rearrange("(c p) n -> p c n", p=128)
                  # per-chunk loads so the first matmuls start early
                  for c in range(kch):
                      nc.sync.dma_start(xT_sb[:, c, :], xT_r[:, c, :])
                      nc.scalar.dma_start(w1_sb[:, c, :], W1_r[:, c, :])
                  for t in range(ntile):
                      ps = apsum.tile([128, w1c], F32, tag="ps")
                      ns = t * 128
                      for c in range(kch):
                          lhsT = xT_sb[:, c, ns : ns + 128]
                          nc.tensor.matmul(
                              ps[:, 0:512], lhsT, w1_sb[:, c, 0:512],
                              start=(c == 0), stop=(c == kch - 1),
                          )
                          nc.tensor.matmul(
                              ps[:, 512:w1c], lhsT, w1_sb[:, c, 512:w1c],
                              start=(c == 0), stop=(c == kch - 1),
                          )
                      hrow = astage.tile([128, ROW1], HD, tag="hrow")
                      nc.scalar.copy(hrow[:, 0:C1], ps[:, 0:C1])
                      av = (hrow[:, C1 : C1 + 48].bitcast(BF16) if FP8
                            else hrow[:, C1 : C1 + 24])
                      nc.vector.tensor_copy(
                          av[:, 0:12], ps[:, C1 + 64 : C1 + 76]
                      )
                      nc.vector.tensor_tensor(
                          av[:, 12:24],
                          ps[:, C1 + 64 : C1 + 76],
                          av[:, 0:12], mybir.AluOpType.subtract,
                      )
                      nc.vector.memset(
                          hrow[:, C1 + (48 if FP8 else 24) : ROW1], 0.0)
                      nc.sync.dma_start(h1_stage[ns : ns + 128, :], hrow[:])
                      # skip = relu(x@Wskip + b_skip)
                      tsk = astage.tile([128, 64], F32, tag="tsk")
                      nc.vector.tensor_tensor(
                          tsk[:], ps[:, C1 : C1 + 64], cb[:, 960:1024],
                          mybir.AluOpType.add,
                      )
                      nc.vector.tensor_scalar_max(skip_sb[:, t, :], tsk[:], 0.0)

              _mark(nc, "A2:AG-H1")
              if stage_cap >= 2:
                  nc.gpsimd.collective_compute(
                      "AllGather", mybir.AluOpType.bypass, replica_groups=rg,
                      ins=[h1_stage.opt()], outs=[H1full.opt()],
                  )

              # =============== helper: GAT aggregation stage ==================
              def gat_agg(row, ch, nh, b_off, g_off, be_off,
                          out_T, src_full_ap):
                  """Per dst-block: gather rows, softmax-weighted segment sum,
                  head mean + bias + LN + relu; writes [nodes,128] bf16 blocks
                  transposed into out_T."""
                  kmaxb = int(max(Kb))
                  with (
                      tc.tile_pool(name=f"g{nh}", bufs=GB) as gp,
                      tc.tile_pool(name=f"gd{nh}", bufs=3) as gdp,
                      tc.tile_pool(name=f"m{nh}", bufs=2) as mp,
                      tc.tile_pool(name=f"s{nh}", bufs=2) as sp,
                      tc.tile_pool(name=f"sml{nh}", bufs=3) as sml,
                      tc.tile_pool(name=f"po{nh}", bufs=POB, space="PSUM") as pop,
                      tc.tile_pool(name=f"pt{nh}", bufs=2, space="PSUM") as ptp,
                  ):
                      for b in range(nblk):
                          K = int(Kb[b])
                          co = int(chunk_off[b])
                          G = gp.tile([128, kmaxb, row], HD, tag="G")
                          for c0 in range(0, K, 8):
                              kk = min(8, K - c0)
                              nc.gpsimd.dma_gather(
                                  G[:, c0 : c0 + kk, :], src_full_ap,
                                  idxs[:, (co + c0) * 8 : (co + c0 + kk) * 8],
                                  num_idxs=kk * 128, num_idxs_reg=kk * 128,
                                  elem_size=row, elem_step=row,
                              )
                          # alpha block of the DST rows (256B window at col ch)
                          wcols = 256 if FP8 else 128
                          Gd = gdp.tile([128, kmaxb, wcols], HD, tag="Gd")
                          for c0 in range(0, K, 8):
                              kk = min(8, K - c0)
                              nc.gpsimd.dma_gather(
                                  Gd[:, c0 : c0 + kk, :],
                                  src_full_ap[:, ch : ch + wcols],
                                  idxsd[:, (co + c0) * 8 : (co + c0 + kk) * 8],
                                  num_idxs=kk * 128, num_idxs_reg=kk * 128,
                                  elem_size=wcols, elem_step=row,
                              )
                          # bf16 views of the alpha cols (raw bytes when fp8)
                          Gav = (G[:, :, ch : ch + 8 * nh].bitcast(BF16)
                                 if FP8 else G[:, :, ch : ch + 4 * nh])
                          Gdav = (Gd[:, :, 0 : 8 * nh].bitcast(BF16)
                                  if FP8 else Gd[:, :, 0 : 4 * nh])
                          if agg_cap < 2:
                              continue
                          s01_sb = sp.tile([128, kmaxb, 128], BF16, tag="s01")
                          (nc.sync if S01SYNC else nc.scalar).dma_start(
                              s01_sb[:, 0:K, :], s01b[:, co * 128 : (co + K) * 128]
                          )
                          if agg_cap < 4:
                              continue
                          # alpha = lrelu(asrc_src + adst_dst); e = exp(alpha)
                          # processed in sub-groups of GSZ chunks so PE can
                          # start aggregating while DVE scales later groups
                          al = sml.tile([128, kmaxb, nh], F32, tag="al")
                          e_sb = sml.tile([128, kmaxb, nh], F32, tag="e")
                          e_dup = sml.tile([128, kmaxb, nh, 2], BF16,
                                           tag="edup")
                          M = mp.tile([128, kmaxb, ch + 8], BF16, tag="M")
                          po = pop.tile([128, ch + 8], F32, tag="po")
                          # sub-group size GSZ from cfg
                          for g0 in range(0, K, GSZ):
                              g1 = min(K, g0 + GSZ)
                              gs = slice(g0, g1)
                              gn_ = g1 - g0
                              nc.vector.tensor_tensor(
                                  al[:, gs, :], Gav[:, gs, 0:nh],
                                  Gav[:, gs, 2 * nh : 3 * nh],
                                  mybir.AluOpType.add,
                              )
                              nc.vector.tensor_tensor(
                                  al[:, gs, :], al[:, gs, :],
                                  Gdav[:, gs, nh : 2 * nh],
                                  mybir.AluOpType.add,
                              )
                              nc.vector.tensor_tensor(
                                  al[:, gs, :], al[:, gs, :],
                                  Gdav[:, gs, 3 * nh : 4 * nh],
                                  mybir.AluOpType.add,
                              )
                              nc.vector.scalar_tensor_tensor(
                                  al[:, gs, :], al[:, gs, :], 0.2,
                                  al[:, gs, :],
                                  mybir.AluOpType.mult, mybir.AluOpType.max,
                              )
                              if agg_cap < 5:
                                  continue
                              nc.scalar.activation(
                                  e_sb[:, gs, :], al[:, gs, :],
                                  mybir.ActivationFunctionType.Exp,
                              )
                              nc.vector.tensor_copy(
                                  M[:, gs, ch : ch + nh], e_sb[:, gs, :]
                              )
                              # scaled messages; pair-duplicated e keeps the
                              # DVE tensor_tensor in 2x_1P mode
                              nc.vector.tensor_copy(
                                  e_dup[:, gs],
                                  e_sb[:, gs].unsqueeze(3).broadcast_to(
                                      [128, gn_, nh, 2]
                                  ),
                              )
                              if FP8 and not MIXED:
                                  # upconvert on ACT, scale in place on DVE
                                  nc.scalar.copy(
                                      M[:, gs, 0:ch], G[:, gs, 0:ch]
                                  )
                                  min_ = M[:, gs, 0:ch]
                              else:
                                  # bf16, or mixed fp8xbf16 DVE read
                                  min_ = G[:, gs, 0:ch]
                              nc.vector.tensor_tensor(
                                  M[:, gs, 0:ch].rearrange(
                                      "p k (h q t) -> p k h q t", h=nh, t=2
                                  ),
                                  min_.rearrange(
                                      "p k (h q t) -> p k h q t", h=nh, t=2
                                  ),
                                  e_dup[:, gs].unsqueeze(3).broadcast_to(
                                      [128, gn_, nh, 64, 2]
                                  ),
                                  mybir.AluOpType.mult,
                              )
                              if agg_cap < 6:
                                  continue
                              # aggregate (last nh cols accumulate the
                              # denominators)
                              for c in range(g0, g1):
                                  first, last = c == 0, c == K - 1
                                  nc.tensor.matmul(
                                      po[:, 0:512], s01_sb[:, c, :],
                                      M[:, c, 0:512],
                                      start=first, stop=last,
                                  )
                                  nc.tensor.matmul(
                                      po[:, 512 : ch + nh], s01_sb[:, c, :],
                                      M[:, c, 512 : ch + nh],
                                      start=first, stop=last,
                                  )
                          if agg_cap < 5 or agg_cap < 6:
                              continue
                          if agg_cap < 61:
                              continue
                          # normalize + head mean + bias + LN + relu
                          den = sml.tile([128, nh], F32, tag="den")
                          nc.vector.tensor_scalar_max(
                              den[:], po[:, ch : ch + nh], 1e-30
                          )
                          rden = sml.tile([128, nh], F32, tag="rden")
                          nc.vector.reciprocal(rden[:], den[:])
                          if agg_cap < 62:
                              continue
                          gn = sml.tile([128, ch], F32, tag="gn")
                          nc.vector.tensor_tensor(
                              gn.rearrange("p (h c) -> p h c", c=128),
                              po[:, 0:ch].rearrange("p (h c) -> p h c", c=128),
                              rden.unsqueeze(2).broadcast_to([128, nh, 128]),
                              mybir.AluOpType.mult,
                          )
                          if agg_cap < 63:
                              continue
                          hm = sml.tile([128, 128], F32, tag="hm")
                          if nh == 6:
                              t2 = sml.tile([128, 384], F32, tag="t2")
                              nc.vector.tensor_tensor(
                                  t2[:], gn[:, 0:384], gn[:, 384:768],
                                  mybir.AluOpType.add,
                              )
                              nc.vector.tensor_tensor(
                                  hm[:], t2[:, 0:128], t2[:, 128:256],
                                  mybir.AluOpType.add,
                              )
                              nc.vector.tensor_tensor(
                                  hm[:], hm[:], t2[:, 256:384],
                                  mybir.AluOpType.add,
                              )
                          else:
                              t2 = sml.tile([128, 256], F32, tag="t2")
                              nc.vector.tensor_tensor(
                                  t2[:], gn[:, 0:256], gn[:, 256:512],
                                  mybir.AluOpType.add,
                              )
                              nc.vector.tensor_tensor(
                                  hm[:], t2[:, 0:128], t2[:, 128:256],
                                  mybir.AluOpType.add,
                              )
                          # hm = hm/nh + bias
                          nc.vector.scalar_tensor_tensor(
                              hm[:], hm[:], 1.0 / nh, cb[:, b_off : b_off + 128],
                              mybir.AluOpType.mult, mybir.AluOpType.add,
                          )
                          if agg_cap < 64:
                              continue
                          # LayerNorm over 128
                          nsum = sml.tile([128, 1], F32, tag="nsum")
                          nc.vector.tensor_reduce(
                              nsum[:], hm[:], mybir.AxisListType.X,
                              mybir.AluOpType.add, negate=True,
                          )
                          nmu = sml.tile([128, 1], F32, tag="nmu")
                          nc.scalar.mul(nmu[:], nsum[:], 1.0 / 128)
                          xc = sml.tile([128, 128], F32, tag="xc")
                          nc.vector.tensor_scalar_add(xc[:], hm[:], nmu[:])
                          if agg_cap < 65:
                              continue
                          sq = sml.tile([128, 128], F32, tag="sq")
                          ss = sml.tile([128, 1], F32, tag="ss")
                          nc.vector.tensor_tensor(
                              sq[:], xc[:], xc[:], mybir.AluOpType.mult
                          )
                          nc.vector.tensor_reduce(
                              ss[:], sq[:], mybir.AxisListType.X,
                              mybir.AluOpType.add,
                          )
                          if agg_cap < 66:
                              continue
                          sd = sml.tile([128, 1], F32, tag="sd")
                          nc.scalar.activation(
                              sd[:], ss[:], mybir.ActivationFunctionType.Sqrt,
                              bias=epsb[:], scale=1.0 / 128,
                          )
                          rstd = sml.tile([128, 1], F32, tag="rstd")
                          nc.vector.reciprocal(rstd[:], sd[:])
                          if agg_cap < 67:
                              continue
                          t3 = sml.tile([128, 128], F32, tag="t3")
                          nc.vector.scalar_tensor_tensor(
                              t3[:], xc[:], rstd[:], cb[:, g_off : g_off + 128],
                              mybir.AluOpType.mult, mybir.AluOpType.mult,
                          )
                          t4 = sml.tile([128, 128], F32, tag="t4")
                          nc.vector.tensor_tensor(
                              t4[:], t3[:], cb[:, be_off : be_off + 128],
                              mybir.AluOpType.add,
                          )
                          xg = sml.tile([128, 128], BF16, tag="xg")
                          nc.vector.tensor_scalar_max(xg[:], t4[:], 0.0)
                          if agg_cap < 68:
                              continue
                          # transpose for the next GEMM
                          pt = ptp.tile([128, 128], BF16, tag="pt")
                          nc.tensor.transpose(pt[:], xg[:], idt[:])
                          nc.vector.tensor_copy(out_T[:, b, :], pt[:])

              # =============== Stage B: GAT1 aggregation ======================
              _mark(nc, "B:gat1-agg")
              x1gat_T = pp.tile([128, ntile, 128], BF16)
              if stage_cap >= 3:
                  gat_agg(ROW1, C1, H1, 0, 128, 256, x1gat_T,
                          H1full[:])

              # =============== Stage C: GCN1 gemm + AG ========================
              _mark(nc, "C:gcn1-gemm+AG")
              if stage_cap >= 4:
                with (
                  tc.tile_pool(name="cps", bufs=2, space="PSUM") as cps,
                  tc.tile_pool(name="cst", bufs=3) as cst,
              ):
                  for t in range(ntile):
                      ps = cps.tile([128, 128], F32, tag="cps")
                      nc.tensor.matmul(ps[:], x1gat_T[:, t, :], wgcn1_sb[:],
                                       start=True, stop=True)
                      yr = cst.tile([128, 128], BF16, tag="yr")
                      nc.scalar.copy(yr[:], ps[:])
                      nc.sync.dma_start(y1_stage[t * 128 : (t + 1) * 128, :], yr[:])
                nc.gpsimd.collective_compute(
                    "AllGather", mybir.AluOpType.bypass, replica_groups=rg,
                    ins=[y1_stage.opt()], outs=[Y1full.opt()],
                )

              # =============== Stage D: GCN1 aggregation ======================
              def gcn_agg(Yfull_ap, ccols, b_off, out_T, out_rows):
                  kmaxb = int(max(Kb))
                  with (
                      tc.tile_pool(name="gy", bufs=3) as gyp,
                      tc.tile_pool(name="sgp", bufs=3) as sgp,
                      tc.tile_pool(name="dsm", bufs=3) as dsm,
                      tc.tile_pool(name="dpo", bufs=2, space="PSUM") as dpo,
                      tc.tile_pool(name="dpt", bufs=2, space="PSUM") as dpt,
                  ):
                      for b in range(nblk):
                          K = int(Kb[b])
                          co = int(chunk_off[b])
                          Gy = gyp.tile([128, kmaxb, 128], BF16, tag="Gy")
                          for c0 in range(0, K, 8):
                              kk = min(8, K - c0)
                              nc.gpsimd.dma_gather(
                                  Gy[:, c0 : c0 + kk, :], Yfull_ap,
                                  idxs[:, (co + c0) * 8 : (co + c0 + kk) * 8],
                                  num_idxs=kk * 128, num_idxs_reg=kk * 128,
                                  elem_size=128, elem_step=128,
                              )
                          sg_sb = sgp.tile([128, kmaxb, 128], BF16, tag="sg")
                          nc.scalar.dma_start(
                              sg_sb[:, 0:K, :], sgb[:, co * 128 : (co + K) * 128]
                          )
                          po = dpo.tile([128, ccols], F32, tag="dpo")
                          for c in range(K):
                              nc.tensor.matmul(
                                  po[:], sg_sb[:, c, :], Gy[:, c, 0:ccols],
                                  start=(c == 0), stop=(c == K - 1),
                              )
                          t5 = dsm.tile([128, ccols], F32, tag="t5")
                          nc.vector.tensor_tensor(
                              t5[:], po[:], cb[:, b_off : b_off + ccols],
                              mybir.AluOpType.add,
                          )
                          xg = dsm.tile([128, ccols], BF16, tag="xgc")
                          nc.vector.tensor_scalar_max(xg[:], t5[:], 0.0)
                          if out_rows is not None:
                              nc.vector.tensor_copy(out_rows[:, b, :], xg[:])
                          pt = dpt.tile([128, 128], BF16, tag="dpt")
                          nc.tensor.transpose(
                              pt[0:ccols, 0:128], xg[:, 0:ccols], idt[:]
                          )
                          nc.vector.tensor_copy(
                              out_T[0:ccols, b, :], pt[0:ccols, 0:128]
                          )

              _mark(nc, "D:gcn1-agg")
              if stage_cap >= 5:
                  gcn_agg(Y1full[:], 128, 384, x1gcn_T, None)

              # =============== Stage E: GAT2 gemm + AG ========================
              _mark(nc, "E:gat2-gemm+AG")
              if stage_cap >= 6:
                with (
                  tc.tile_pool(name="eps", bufs=2, space="PSUM") as epsp,
                  tc.tile_pool(name="est", bufs=3) as est,
              ):
                  for t in range(ntile):
                      ps = epsp.tile([128, 520], F32, tag="eps")
                      nc.tensor.matmul(ps[:, 0:512], x1gcn_T[:, t, :],
                                       w2_sb[:, 0:512], start=True, stop=True)
                      nc.tensor.matmul(ps[:, 512:520], x1gcn_T[:, t, :],
                                       w2_sb[:, 512:520], start=True, stop=True)
                      hrow = est.tile([128, ROW2], HD, tag="hrow2")
                      nc.scalar.copy(hrow[:, 0:C2], ps[:, 0:C2])
                      av2 = (hrow[:, C2 : C2 + 32].bitcast(BF16) if FP8
                             else hrow[:, C2 : C2 + 16])
                      nc.vector.tensor_copy(av2[:, 0:8], ps[:, 512:520])
                      nc.vector.tensor_tensor(
                          av2[:, 8:16], ps[:, 512:520],
                          av2[:, 0:8], mybir.AluOpType.subtract,
                      )
                      nc.vector.memset(
                          hrow[:, C2 + (32 if FP8 else 16) : ROW2], 0.0)
                      nc.sync.dma_start(h2_stage[t * 128 : (t + 1) * 128, :], hrow[:])
                nc.gpsimd.collective_compute(
                    "AllGather", mybir.AluOpType.bypass, replica_groups=rg,
                    ins=[h2_stage.opt()], outs=[H2full.opt()],
                )

              # =============== Stage F: GAT2 aggregation ======================
              _mark(nc, "F:gat2-agg")
              x2gat_T = pp.tile([128, ntile, 128], BF16)
              if stage_cap >= 7:
                  gat_agg(ROW2, C2, H2, 512, 640, 768, x2gat_T,
                          H2full[:])

              # =============== Stage G: GCN2 gemm + AG ========================
              _mark(nc, "G:gcn2-gemm+AG")
              if stage_cap >= 8:
                with (
                  tc.tile_pool(name="gps", bufs=2, space="PSUM") as gps,
                  tc.tile_pool(name="gst", bufs=3) as gst,
              ):
                  for t in range(ntile):
                      ps = gps.tile([128, 64], F32, tag="gps")
                      nc.tensor.matmul(ps[:], x2gat_T[:, t, :], wgcn2_sb[:],
                                       start=True, stop=True)
                      yr = gst.tile([128, 128], BF16, tag="yr2")
                      nc.scalar.copy(yr[:, 0:64], ps[:])
                      nc.vector.memset(yr[:, 64:128], 0.0)
                      nc.sync.dma_start(y2_stage[t * 128 : (t + 1) * 128, :], yr[:])
                nc.gpsimd.collective_compute(
                    "AllGather", mybir.AluOpType.bypass, replica_groups=rg,
                    ins=[y2_stage.opt()], outs=[Y2full.opt()],
                )

              # =============== Stage H: GCN2 aggregation ======================
              _mark(nc, "H:gcn2-agg")
              x2gcn_rows = pp.tile([128, ntile, 64], BF16)
              if stage_cap >= 9:
                  gcn_agg(Y2full[:], 64, 896, x2gcn_T, x2gcn_rows)

              # =============== Stage I: fuse + LN3 + classifier ===============
              _mark(nc, "I:fuse+clf")
              if stage_cap >= 10:
                with (
                  tc.tile_pool(name="ips", bufs=2, space="PSUM") as ips,
                  tc.tile_pool(name="ipt", bufs=2, space="PSUM") as ipt,
                  tc.tile_pool(name="icl", bufs=1, space="PSUM") as icl,
                  tc.tile_pool(name="ism", bufs=3) as ism,
                  tc.tile_pool(name="ift", bufs=1) as ift,
              ):
                  fT_all = ift.tile([64, ntile, 128], BF16)
                  for t in range(ntile):
                      pf = ips.tile([128, 64], F32, tag="ip")
                      nc.tensor.matmul(pf[:], x1gcn_T[:, t, :], wf_sb[:, 0, :],
                                       start=True, stop=False)
                      nc.tensor.matmul(pf[:], x2gcn_T[0:64, t, :],
                                       wf_sb[0:64, 1, :], start=False, stop=True)
                      tf = ism.tile([128, 64], F32, tag="tf")
                      nc.vector.tensor_tensor(
                          tf[:], pf[:], cb[:, 1024:1088], mybir.AluOpType.add
                      )
                      nc.vector.tensor_scalar_max(tf[:], tf[:], 0.0)
                      nc.vector.tensor_tensor(
                          tf[:], tf[:], skip_sb[:, t, :], mybir.AluOpType.add
                      )
                      # LN3 over 64
                      nsum = ism.tile([128, 1], F32, tag="insum")
                      nc.vector.tensor_reduce(
                          nsum[:], tf[:], mybir.AxisListType.X,
                          mybir.AluOpType.add, negate=True,
                      )
                      nmu = ism.tile([128, 1], F32, tag="inmu")
                      nc.scalar.mul(nmu[:], nsum[:], 1.0 / 64)
                      xc = ism.tile([128, 64], F32, tag="ixc")
                      nc.vector.tensor_scalar_add(xc[:], tf[:], nmu[:])
                      sq = ism.tile([128, 64], F32, tag="isq")
                      ss = ism.tile([128, 1], F32, tag="iss")
                      nc.vector.tensor_tensor(
                          sq[:], xc[:], xc[:], mybir.AluOpType.mult
                      )
                      nc.vector.tensor_reduce(
                          ss[:], sq[:], mybir.AxisListType.X,
                          mybir.AluOpType.add,
                      )
                      sd = ism.tile([128, 1], F32, tag="isd")
                      nc.scalar.activation(
                          sd[:], ss[:], mybir.ActivationFunctionType.Sqrt,
                          bias=epsb[:], scale=1.0 / 64,
                      )
                      rstd = ism.tile([128, 1], F32, tag="irstd")
                      nc.vector.reciprocal(rstd[:], sd[:])
                      t3 = ism.tile([128, 64], F32, tag="it3")
                      nc.vector.scalar_tensor_tensor(
                          t3[:], xc[:], rstd[:], cb[:, 1088:1152],
                          mybir.AluOpType.mult, mybir.AluOpType.mult,
                      )
                      fin = ism.tile([128, 64], BF16, tag="fin")
                      nc.vector.tensor_tensor(
                          fin[:], t3[:], cb[:, 1152:1216], mybir.AluOpType.add
                      )
                      # stash transposed fin for the batched classifier
                      ptr = ipt.tile([128, 128], BF16, tag="ptr")
                      nc.tensor.transpose(ptr[0:64, 0:128], fin[:, 0:64], idt[:])
                      nc.vector.tensor_copy(fT_all[:, t, :], ptr[0:64, 0:128])
                  # batched classifier in transposed space, 512-col slices:
                  # relu(Wc1^T fT + b) -> relu(Wc2^T . + b) -> Wc3^T . + b
                  NCOLS = ntile * 128
                  fT_f = fT_all.rearrange("p t n -> p (t n)")
                  orow = ism.tile([8, NCOLS], F32, tag="orow")
                  for c0 in range(0, NCOLS, 512):
                      cs = slice(c0, min(NCOLS, c0 + 512))
                      cw = cs.stop - c0
                      p1 = icl.tile([32, 512], F32, tag="p1")
                      nc.tensor.matmul(p1[:, 0:cw], wc1_sb[:], fT_f[:, cs],
                                       start=True, stop=True)
                      h1b = ism.tile([32, 512], BF16, tag="bh1")
                      nc.vector.tensor_scalar(
                          h1b[:, 0:cw], p1[:, 0:cw], cbT_sb[0:32, 0:1], 0.0,
                          mybir.AluOpType.add, mybir.AluOpType.max,
                      )
                      p2 = icl.tile([16, 512], F32, tag="p2")
                      nc.tensor.matmul(p2[:, 0:cw], wc2_sb[:], h1b[:, 0:cw],
                                       start=True, stop=True)
                      h2b = ism.tile([16, 512], BF16, tag="bh2")
                      nc.vector.tensor_scalar(
                          h2b[:, 0:cw], p2[:, 0:cw], cbT_sb[0:16, 1:2], 0.0,
                          mybir.AluOpType.add, mybir.AluOpType.max,
                      )
                      p3 = icl.tile([8, 512], F32, tag="p3")
                      nc.tensor.matmul(p3[:, 0:cw], wc3_sb[:], h2b[:, 0:cw],
                                       start=True, stop=True)
                      nc.vector.tensor_scalar_add(
                          orow[:, cs], p3[:, 0:cw], cbT_sb[0:8, 2:3]
                      )
                  nc.sync.dma_start(out[:, :], orow[:])

            _mark(nc, "Z:end")
            if stage_cap < 10:
                with tc.tile_pool(name="fb", bufs=1) as fb:
                    z = fb.tile([128, 8], F32)
                    nc.vector.memset(z[:], 0.0)
                    for t in range(ntile):
                        nc.sync.dma_start(out[t * 128 : (t + 1) * 128, :], z[:])

    nc.compile()
    return nc


# ----------------------------------------------------------------------------
# Top-level kernel
# ----------------------------------------------------------------------------

_CACHE = {}
STAGE_MARKS = []


def _mark(nc, label):
    try:
        STAGE_MARKS.append((label, int(nc.next_id())))
    except Exception:
        pass


def prepare(inputs, n_nodes=None, stage_cap=99, agg_cap=99, repeat=1,
            fp8=True, gsz=6, mixed=False, gbufs=2, s01sync=False, pobufs=2):
    """Host prep + (cached) program build. Returns (nc, in_maps, n_loc)."""
    x = np.asarray(inputs["x"], np.float32)
    n = x.shape[0] if n_nodes is None else n_nodes
    f_in = x.shape[1]
    assert n % NCORES == 0
    n_loc = n // NCORES
    n_loc_pad = _rup(n_loc, 128)
    k_pad = _rup(f_in, 128)

    g = prep_graph(inputs["edge_index"], n, n_loc, n_loc_pad)
    w = prep_weights(inputs, k_pad)

    cfg_key = (n_loc_pad, k_pad, w["w1c"], tuple(g["K"]), stage_cap, agg_cap,
               repeat, fp8, gsz, mixed, gbufs, s01sync, pobufs)
    if cfg_key not in _CACHE:
        cfg = {
            "n_loc_pad": n_loc_pad,
            "k_pad": k_pad,
            "w1c": w["w1c"],
            "K": g["K"],
            "totch": g["totch"],
            "nidx": g["nidx"],
            "chunk_off": g["chunk_off"],
            "stage_cap": stage_cap,
            "agg_cap": agg_cap,
            "repeat": repeat,
            "fp8": fp8,
            "gsz": gsz,
            "mixed": mixed,
            "gbufs": gbufs,
            "s01sync": s01sync,
            "pobufs": pobufs,
        }
        _CACHE[cfg_key] = build_nc(cfg)
    nc = _CACHE[cfg_key]

    xp = np.zeros((NCORES * n_loc_pad, k_pad), np.float32)
    for c in range(NCORES):
        xp[c * n_loc_pad : c * n_loc_pad + n_loc, :f_in] = x[
            c * n_loc : (c + 1) * n_loc
        ]
    xpb = xp.astype(NPBF16)

    in_maps = []
    for c in range(NCORES):
        xT_loc = np.ascontiguousarray(
            xpb[c * n_loc_pad : (c + 1) * n_loc_pad].T
        )
        in_maps.append(
            {
                "xT": xT_loc,
                "W1": w["W1"],
                "W2": w["W2"],
                "Wgcn1": w["Wgcn1"],
                "Wgcn2": w["Wgcn2"],
                "Wfuse": w["Wfuse"],
                "Wc1": w["Wc1"],
                "Wc2": w["Wc2"],
                "Wc3": w["Wc3"],
                "cblob": w["cblob"],
                "cbT": w["cbT"],
                "ident": w["ident"],
                "idx16": g["idx16"][c],
                "idx16d": g["idx16d"][c],
                "s01": g["s01"][c],
                "sg": g["sg"][c],
            }
        )
    return nc, in_maps, n_loc, n_loc_pad


def kernel(**inputs):
    nc, in_maps, n_loc, n_loc_pad = prepare(inputs)
    res = run_bass_kernel_spmd(nc, in_maps, core_ids=list(range(NCORES)))
    n = np.asarray(inputs["x"]).shape[0]
    out = np.zeros((n, 5), np.float32)
    for c in range(NCORES):
        out[c * n_loc : (c + 1) * n_loc] = res.results[c]["out"][:5, :n_loc].T
    return out


if __name__ == "__main__":
    # quick smoke: tiny random problem shaped like the real one
    rng = np.random.default_rng(0)
    N, E, F_IN = 256, 2048, 96
    ip = {
        "x": rng.standard_normal((N, F_IN), dtype=np.float32),
        "edge_index": rng.integers(0, N, (2, E)),
        "W_gat1": rng.standard_normal((F_IN, 768), dtype=np.float32) * 0.05,
        "a_src1": rng.standard_normal((6, 128), dtype=np.float32) * 0.05,
        "a_dst1": rng.standard_normal((6, 128), dtype=np.float32) * 0.05,
        "b_gat1": np.zeros(128, np.float32),
        "W_gcn1": rng.standard_normal((128, 128), dtype=np.float32) * 0.05,
        "b_gcn1": np.zeros(128, np.float32),
        "W_gat2": rng.standard_normal((128, 512), dtype=np.float32) * 0.05,
        "a_src2": rng.standard_normal((4, 128), dtype=np.float32) * 0.05,
        "a_dst2": rng.standard_normal((4, 128), dtype=np.float32) * 0.05,
        "b_gat2": np.zeros(128, np.float32),
        "W_gcn2": rng.standard_normal((128, 64), dtype=np.float32) * 0.05,
        "b_gcn2": np.zeros(64, np.float32),
        "W_skip": rng.standard_normal((F_IN, 64), dtype=np.float32) * 0.05,
        "b_skip": np.zeros(64, np.float32),
        "W_fuse": rng.standard_normal((192, 64), dtype=np.float32) * 0.05,
        "b_fuse": np.zeros(64, np.float32),
        "W_c1": rng.standard_normal((64, 32), dtype=np.float32) * 0.05,
        "b_c1": np.zeros(32, np.float32),
        "W_c2": rng.standard_normal((32, 16), dtype=np.float32) * 0.05,
        "b_c2": np.zeros(16, np.float32),
        "W_c3": rng.standard_normal((16, 5), dtype=np.float32) * 0.05,
        "b_c3": np.zeros(5, np.float32),
        "g1": np.ones(128, np.float32), "be1": np.zeros(128, np.float32),
        "g2": np.ones(128, np.float32), "be2": np.zeros(128, np.float32),
        "g3": np.ones(64, np.float32), "be3": np.zeros(64, np.float32),
    }
    t0 = time.time()
    outv = kernel(**ip)
    print("kernel ran in", time.time() - t0, "shape", outv.shape)
    print(outv[:4])



# revision 18
# speedup vs baseline: 1.5650x; 1.5650x over previous
"""Trainium2 Bass kernel for nn_AdvancedCardiomyocyteGNN (GAT/GCN message passing).

Strategy (8 NeuronCores, SPMD single NEFF):
  - Nodes sharded across cores (1250 -> padded 1280 per core).
  - Node-wise GEMMs computed on the owning core; per-edge alpha projections
    (h . a_src / h . a_dst) are folded into the main GEMM weights on the host.
  - AllGather replicates the transformed node features (bf16) to all cores.
  - Edges partitioned by dst, sorted, grouped per 128-dst block; source rows
    are fetched with dma_gather (128 edges per chunk land on 128 partitions);
    segment softmax + weighted segment sum are computed as one-hot matmuls
    (S^T @ M accumulated in PSUM per dst block).
  - Graph-structure-dependent one-hot/scatter matrices and index tables are
    precomputed on the host (pure preprocessing of the integer edge list).
"""

import sys
import time

sys.path.insert(0, "/opt/trn_rl_repo")

import numpy as np
import ml_dtypes

import concourse.bass as bass
import concourse.tile as tile
from concourse import bacc, mybir
from concourse.bass_utils import run_bass_kernel_spmd

F32 = mybir.dt.float32
BF16 = mybir.dt.bfloat16
F8 = mybir.dt.float8e4
I16 = mybir.dt.int16
NPBF16 = ml_dtypes.bfloat16

NCORES = 8


def _rup(x, m):
    return (x + m - 1) // m * m


# ----------------------------------------------------------------------------
# Host-side graph preprocessing
# ----------------------------------------------------------------------------

def prep_graph(edge_index, n_nodes, n_loc, n_loc_pad, heads_dummy=None):
    """Partition edges (with self loops) by dst across cores, sort by dst,
    group per 128-dst block, pad each block to a per-block common chunk count.

    Returns dict with per-core index/scatter data and layout constants."""
    src = np.asarray(edge_index[0], dtype=np.int64)
    dst = np.asarray(edge_index[1], dtype=np.int64)
    loop = np.arange(n_nodes, dtype=np.int64)
    src = np.concatenate([src, loop])
    dst = np.concatenate([dst, loop])

    # gcn normalization (reference: deg over dst including self loops)
    deg = np.bincount(dst, minlength=n_nodes).astype(np.float64)
    dinv = np.where(deg > 0, deg ** -0.5, 0.0)
    ce_all = (dinv[src] * dinv[dst]).astype(np.float32)

    # padded node ids
    def pad_id(n):
        return (n // n_loc) * n_loc_pad + (n % n_loc)

    srcp = pad_id(src)
    dstp = pad_id(dst)

    core_of = dst // n_loc
    nblk = n_loc_pad // 128

    # per (core, blk) edge lists
    per_core = []
    for c in range(NCORES):
        m = core_of == c
        s, d, ce = srcp[m], dstp[m], ce_all[m]
        dloc = d - c * n_loc_pad
        order = np.argsort(dloc, kind="stable")
        s, dloc, ce = s[order], dloc[order], ce[order]
        blk = dloc // 128
        per_core.append((s, dloc, ce, blk))

    # per-block chunk count, common across cores
    K = np.zeros(nblk, dtype=np.int64)
    for c in range(NCORES):
        _, _, _, blk = per_core[c]
        cnt = np.bincount(blk, minlength=nblk)
        K = np.maximum(K, (cnt + 127) // 128)
    K = np.maximum(K, 1).astype(int)
    totch = int(K.sum())
    nidx = totch * 128

    idx16 = np.zeros((NCORES, 128, nidx // 16), dtype=np.int16)
    s01 = np.zeros((NCORES, 128, totch * 128), dtype=NPBF16)
    s01T = np.zeros((NCORES, 128, totch * 128), dtype=NPBF16)
    sg = np.zeros((NCORES, 128, totch * 128), dtype=NPBF16)

    chunk_off = np.concatenate([[0], np.cumsum(K)])  # chunk offset per block

    for c in range(NCORES):
        s, dloc, ce, blk = per_core[c]
        idx_flat = np.zeros(nidx, dtype=np.int16)
        for b in range(nblk):
            m = blk == b
            sb_, db_, cb_ = s[m], dloc[m] - b * 128, ce[m]
            ne = len(sb_)
            base = chunk_off[b] * 128  # edge slot offset
            idx_flat[base : base + ne] = sb_.astype(np.int16)
            ch = base // 128 + np.arange(ne) // 128  # global chunk id
            e_in = np.arange(ne) % 128
            s01[c, e_in, ch * 128 + db_] = NPBF16(1.0)
            s01T[c, db_, ch * 128 + e_in] = NPBF16(1.0)
            sg[c, e_in, ch * 128 + db_] = cb_.astype(NPBF16)
        idx16[c] = np.tile(idx_flat.reshape(-1, 16).T, (8, 1))

    return {
        "K": K,
        "totch": totch,
        "nidx": nidx,
        "chunk_off": chunk_off,
        "idx16": idx16,
        "s01": s01,
        "s01T": s01T,
        "sg": sg,
    }


def prep_weights(ip, k_pad):
    """Fold/concat/cast weights on the host. Returns dict of shared arrays."""
    f32 = np.float32
    w_gat1 = np.asarray(ip["W_gat1"], f32)  # [F_IN, 768]
    a_src1 = np.asarray(ip["a_src1"], f32)  # [6, 128]
    a_dst1 = np.asarray(ip["a_dst1"], f32)
    w_skip = np.asarray(ip["W_skip"], f32)  # [F_IN, 64]
    f_in = w_gat1.shape[0]
    h1 = a_src1.shape[0]
    c1 = a_src1.shape[1]
    ws1 = np.einsum("khc,hc->kh", w_gat1.reshape(f_in, h1, c1), a_src1)
    wd1 = np.einsum("khc,hc->kh", w_gat1.reshape(f_in, h1, c1), a_dst1)
    w1 = np.concatenate([w_gat1, w_skip, ws1, wd1], axis=1)  # [F_IN, 844]
    w1c = _rup(w1.shape[1], 16)
    w1p = np.zeros((k_pad, w1c), NPBF16)
    w1p[:f_in, : w1.shape[1]] = w1.astype(NPBF16)

    w_gat2 = np.asarray(ip["W_gat2"], f32)  # [128, 512]
    a_src2 = np.asarray(ip["a_src2"], f32)  # [4, 128]
    a_dst2 = np.asarray(ip["a_dst2"], f32)
    h2 = a_src2.shape[0]
    ws2 = np.einsum("khc,hc->kh", w_gat2.reshape(128, h2, c1), a_src2)
    wd2 = np.einsum("khc,hc->kh", w_gat2.reshape(128, h2, c1), a_dst2)
    w2 = np.concatenate([w_gat2, ws2, wd2], axis=1)  # [128, 520]
    w2p = w2.astype(NPBF16)

    def rep(v, cols=None):
        v = np.asarray(v, f32).reshape(-1)
        if cols is not None:
            vv = np.zeros(cols, f32)
            vv[: len(v)] = v
            v = vv
        return np.tile(v[None, :], (128, 1)).astype(f32)

    cblob = np.concatenate(
        [
            rep(ip["b_gat1"]),   # 0:128
            rep(ip["g1"]),       # 128:256
            rep(ip["be1"]),      # 256:384
            rep(ip["b_gcn1"]),   # 384:512
            rep(ip["b_gat2"]),   # 512:640
            rep(ip["g2"]),       # 640:768
            rep(ip["be2"]),      # 768:896
            rep(ip["b_gcn2"], 64),   # 896:960
            rep(ip["b_skip"], 64),   # 960:1024
            rep(ip["b_fuse"], 64),   # 1024:1088
            rep(ip["g3"], 64),       # 1088:1152
            rep(ip["be3"], 64),      # 1152:1216
            rep(ip["b_c1"], 32),     # 1216:1248
            rep(ip["b_c2"], 16),     # 1248:1264
            rep(ip["b_c3"], 8),      # 1264:1272
        ],
        axis=1,
    )

    wf = np.asarray(ip["W_fuse"], f32)  # [192, 64]
    wc1 = np.asarray(ip["W_c1"], f32)  # [64, 32]
    wc2 = np.asarray(ip["W_c2"], f32)  # [32, 16]
    wc3 = np.asarray(ip["W_c3"], f32)  # [16, 5]
    wc3p = np.zeros((wc3.shape[0], 8), np.float32)
    wc3p[:, : wc3.shape[1]] = wc3

    cbT = np.zeros((128, 4), f32)
    cbT[:32, 0] = np.asarray(ip["b_c1"], f32)
    cbT[:16, 1] = np.asarray(ip["b_c2"], f32)
    cbT[:5, 2] = np.asarray(ip["b_c3"], f32)

    return {
        "W1": w1p,
        "W2": w2p,
        "Wgcn1": np.asarray(ip["W_gcn1"], f32).astype(NPBF16),
        "Wgcn2": np.asarray(ip["W_gcn2"], f32).astype(NPBF16),
        "Wfuse": wf.astype(NPBF16),
        "Wc1": wc1.astype(NPBF16),
        "Wc2": wc2.astype(NPBF16),
        "Wc3": wc3p.astype(NPBF16),
        "cblob": cblob,
        "cbT": cbT,
        "ident": np.eye(128, dtype=NPBF16),
        "w1c": w1c,
    }


# ----------------------------------------------------------------------------
# Bass program builder
# ----------------------------------------------------------------------------

def build_nc(cfg):
    """cfg: dict with n_loc_pad, k_pad (F_IN padded), K (list per block),
    totch, nidx, w1c, h1=6, h2=4."""
    n_loc_pad = cfg["n_loc_pad"]
    k_pad = cfg["k_pad"]
    Kb = cfg["K"]
    totch = cfg["totch"]
    nidx = cfg["nidx"]
    w1c = cfg["w1c"]
    chunk_off = cfg["chunk_off"]
    nblk = n_loc_pad // 128
    ntile = nblk
    kch = k_pad // 128
    NP = NCORES * n_loc_pad
    H1, H2 = 6, 4
    FP8 = bool(cfg.get("fp8", False))
    GSZ = int(cfg.get("gsz", 6))
    MIXED = bool(cfg.get("mixed", False))
    GB = int(cfg.get("gbufs", 2))
    S01SYNC = bool(cfg.get("s01sync", False))
    POB = int(cfg.get("pobufs", 2))
    HD = F8 if FP8 else BF16
    if FP8:
        # fp8 rows: [h fp8 | alpha hi/lo as raw bf16 bytes | pad to 256B]
        ROW1 = _rup(H1 * 128 + 2 * 24, 256)   # 1024
        ROW2 = _rup(H2 * 128 + 2 * 16, 256)   # 768
    else:
        ROW1 = _rup(H1 * 128 + 24, 128)   # h(768)+asrc hi/lo+adst hi/lo
        ROW2 = _rup(H2 * 128 + 16, 128)
    C1 = H1 * 128
    C2 = H2 * 128
    AS1 = C1 + 24   # alpha block end (asrc hi/lo + adst hi/lo)
    AS2 = C2 + 16
    EPS = 1e-5

    stage_cap = cfg.get("stage_cap", 99)
    agg_cap = cfg.get("agg_cap", 99)
    repeat = cfg.get("repeat", 1)
    STAGE_MARKS.clear()
    nc = bacc.Bacc("TRN2", target_bir_lowering=False, debug=False,
                   num_devices=NCORES)

    def din(name, shape, dt):
        return nc.dram_tensor(name, shape, dt, kind="ExternalInput").ap()

    xT = din("xT", [k_pad, n_loc_pad], BF16)
    W1 = din("W1", [k_pad, w1c], BF16)
    W2 = din("W2", [128, 520], BF16)
    Wgcn1 = din("Wgcn1", [128, 128], BF16)
    Wgcn2 = din("Wgcn2", [128, 64], BF16)
    Wfuse = din("Wfuse", [192, 64], BF16)
    Wc1 = din("Wc1", [64, 32], BF16)
    Wc2 = din("Wc2", [32, 16], BF16)
    Wc3 = din("Wc3", [16, 8], BF16)
    cblob = din("cblob", [128, 1272], F32)
    ident = din("ident", [128, 128], BF16)
    idx16 = din("idx16", [128, nidx // 16], I16)
    s01b = din("s01", [128, totch * 128], BF16)
    s01Tb = din("s01T", [128, totch * 128], BF16)
    sgb = din("sg", [128, totch * 128], BF16)

    cbT = din("cbT", [128, 4], F32)
    out = nc.dram_tensor("out", [8, n_loc_pad], F32, kind="ExternalOutput").ap()

    rg = [list(range(NCORES))]

    with tile.TileContext(nc) as tc:
        with (
            tc.tile_pool(name="const", bufs=1) as cpool,
            tc.tile_pool(name="persist", bufs=2) as pp,
            tc.tile_pool(name="xtp", bufs=3) as axp,
            tc.tile_pool(name="dram", bufs=1, space="DRAM") as dpool,
        ):
            # ---- constants / persistent tiles ----
            cb = cpool.tile([128, 1272], F32)
            nc.scalar.dma_start(cb[:], cblob[:])
            idt = cpool.tile([128, 128], BF16)
            nc.scalar.dma_start(idt[:], ident[:])
            idxs = cpool.tile([128, nidx // 16], I16)
            nc.scalar.dma_start(idxs[:], idx16[:])
            wgcn1_sb = cpool.tile([128, 128], BF16)
            nc.scalar.dma_start(wgcn1_sb[:], Wgcn1[:])
            wgcn2_sb = cpool.tile([128, 64], BF16)
            nc.scalar.dma_start(wgcn2_sb[:], Wgcn2[:])
            w2_sb = cpool.tile([128, 520], BF16)
            nc.scalar.dma_start(w2_sb[:], W2[:])
            wf_sb = cpool.tile([128, 2, 64], BF16)
            nc.scalar.dma_start(wf_sb[:, 0, :], Wfuse[0:128, :])
            nc.scalar.dma_start(wf_sb[:64, 1, :], Wfuse[128:192, :])
            wc1_sb = cpool.tile([64, 32], BF16)
            nc.scalar.dma_start(wc1_sb[:], Wc1[:])
            wc2_sb = cpool.tile([32, 16], BF16)
            nc.scalar.dma_start(wc2_sb[:], Wc2[:])
            wc3_sb = cpool.tile([16, 8], BF16)
            nc.scalar.dma_start(wc3_sb[:], Wc3[:])
            epsb = cpool.tile([128, 1], F32)
            nc.vector.memset(epsb[:], EPS)
            cbT_sb = cpool.tile([128, 4], F32)
            nc.scalar.dma_start(cbT_sb[:], cbT[:])
            # weights for GEMM1 live in SBUF across reps
            w1_sb = cpool.tile([128, kch, w1c], BF16)
            W1_r = W1.rearrange("(c p) n -> p c n", p=128)
            for c in range(kch):
                nc.scalar.dma_start(w1_sb[:, c, :], W1_r[:, c, :])

            # =============== Stage A: GEMM1 (x @ [Wgat1|Wskip|Ws|Wd]) =======
            # Software-pipelined emission: stage A (+ its AllGather) of rep
            # r+1 is emitted BEFORE stages B..I of rep r so the per-engine
            # in-order queues can overlap the next rep's GEMM and AG wire
            # time with the current rep's aggregation work.
            def emit_A(_rep):
              _mark(nc, f"A:gemm1 r{_rep}")
              h1_stage = dpool.tile([n_loc_pad, ROW1], HD, tag=f"h1s{_rep}")
              H1full = dpool.tile([NP, ROW1], HD, addr_space="Shared",
                                  tag=f"H1f{_rep}")
              skip_sb = pp.tile([128, ntile, 64], F32, tag="skip")
              with (
                  tc.tile_pool(name="apsum", bufs=1, space="PSUM") as apsum,
                  tc.tile_pool(name="astage", bufs=3) as astage,
              ):
                  xT_r = xT.rearrange("(c p) n -> p c n", p=128)
                  for t in range(ntile):
                      ns = t * 128
                      xt_sb = axp.tile([128, kch, 128], BF16, tag="xt")
                      nc.sync.dma_start(xt_sb[:], xT_r[:, :, ns : ns + 128])
                      ps = apsum.tile([128, w1c], F32, tag="ps")
                      for c in range(kch):
                          lhsT = xt_sb[:, c, :]
                          nc.tensor.matmul(
                              ps[:, 0:512], lhsT, w1_sb[:, c, 0:512],
                              start=(c == 0), stop=(c == kch - 1),
                          )
                          nc.tensor.matmul(
                              ps[:, 512:w1c], lhsT, w1_sb[:, c, 512:w1c],
                              start=(c == 0), stop=(c == kch - 1),
                          )
                      hrow = astage.tile([128, ROW1], HD, tag="hrow")
                      nc.scalar.copy(hrow[:, 0:C1], ps[:, 0:C1])
                      av = (hrow[:, C1 : C1 + 48].bitcast(BF16) if FP8
                            else hrow[:, C1 : C1 + 24])
                      nc.vector.tensor_copy(
                          av[:, 0:12], ps[:, C1 + 64 : C1 + 76]
                      )
                      nc.vector.tensor_tensor(
                          av[:, 12:24],
                          ps[:, C1 + 64 : C1 + 76],
                          av[:, 0:12], mybir.AluOpType.subtract,
                      )
                      nc.vector.memset(
                          hrow[:, C1 + (48 if FP8 else 24) : ROW1], 0.0)
                      nc.sync.dma_start(h1_stage[ns : ns + 128, :], hrow[:])
                      # skip = relu(x@Wskip + b_skip)
                      tsk = astage.tile([128, 64], F32, tag="tsk")
                      nc.vector.tensor_tensor(
                          tsk[:], ps[:, C1 : C1 + 64], cb[:, 960:1024],
                          mybir.AluOpType.add,
                      )
                      nc.vector.tensor_scalar_max(skip_sb[:, t, :], tsk[:], 0.0)

              _mark(nc, "A2:AG-H1")
              if stage_cap >= 2:
                  nc.gpsimd.collective_compute(
                      "AllGather", mybir.AluOpType.bypass, replica_groups=rg,
                      ins=[h1_stage.opt()], outs=[H1full.opt()],
                  )
              return h1_stage, H1full, skip_sb

            if True:
              # =============== helper: GAT aggregation stage ==================
              def gat_agg(row, ch, nh, b_off, g_off, be_off,
                          out_T, src_full_ap, stage_ap):
                  """Per dst-block: gather rows, softmax-weighted segment sum,
                  head mean + bias + LN + relu; writes [nodes,128] bf16 blocks
                  transposed into out_T."""
                  kmaxb = int(max(Kb))
                  with (
                      tc.tile_pool(name=f"g{nh}", bufs=GB) as gp,
                      tc.tile_pool(name=f"gd{nh}", bufs=3) as gdp,
                      tc.tile_pool(name=f"m{nh}", bufs=2) as mp,
                      tc.tile_pool(name=f"s{nh}", bufs=2) as sp,
                      tc.tile_pool(name=f"sml{nh}", bufs=2) as sml,
                      tc.tile_pool(name=f"po{nh}", bufs=POB, space="PSUM") as pop,
                      tc.tile_pool(name=f"pt{nh}", bufs=2, space="PSUM") as ptp,
                      tc.tile_pool(name=f"pa{nh}", bufs=2, space="PSUM") as pap,
                  ):
                      for b in range(nblk):
                          K = int(Kb[b])
                          co = int(chunk_off[b])
                          G = gp.tile([128, kmaxb, row], HD, tag="G")
                          for c0 in range(0, K, 8):
                              kk = min(8, K - c0)
                              nc.gpsimd.dma_gather(
                                  G[:, c0 : c0 + kk, :], src_full_ap,
                                  idxs[:, (co + c0) * 8 : (co + c0 + kk) * 8],
                                  num_idxs=kk * 128, num_idxs_reg=kk * 128,
                                  elem_size=row, elem_step=row,
                              )
                          # dst-alpha block: the dst rows of block b are the
                          # core's OWN stage rows (local, pre-collective);
                          # broadcast dst alpha to edge slots via per-chunk
                          # matmul with the transposed one-hot s01T.
                          acols = 8 * nh if FP8 else 4 * nh
                          adst = gdp.tile([128, acols], HD, tag="adst")
                          nc.sync.dma_start(
                              adst[:],
                              stage_ap[b * 128 : (b + 1) * 128, ch : ch + acols],
                          )
                          adst_bf = (adst.bitcast(BF16) if FP8 else adst)
                          s01T_sb = gdp.tile([128, kmaxb, 128], BF16, tag="s01T")
                          nc.sync.dma_start(
                              s01T_sb[:, 0:K, :],
                              s01Tb[:, co * 128 : (co + K) * 128],
                          )
                          ald = pap.tile([128, kmaxb, 4 * nh], F32, tag="ald")
                          for c in range(K):
                              nc.tensor.matmul(
                                  ald[:, c, :], s01T_sb[:, c, :],
                                  adst_bf[:, 0 : 4 * nh],
                                  start=True, stop=True,
                              )
                          # bf16 views of the alpha cols (raw bytes when fp8)
                          Gav = (G[:, :, ch : ch + 8 * nh].bitcast(BF16)
                                 if FP8 else G[:, :, ch : ch + 4 * nh])
                          Gdav = ald
                          if agg_cap < 2:
                              continue
                          s01_sb = sp.tile([128, kmaxb, 128], BF16, tag="s01")
                          nc.sync.dma_start(
                              s01_sb[:, 0:K, :], s01b[:, co * 128 : (co + K) * 128]
                          )
                          if agg_cap < 4:
                              continue
                          # alpha = lrelu(asrc_src + adst_dst); e = exp(alpha)
                          # processed in sub-groups of GSZ chunks so PE can
                          # start aggregating while DVE scales later groups
                          al = sml.tile([128, kmaxb, nh], F32, tag="al")
                          e_sb = sml.tile([128, kmaxb, nh], F32, tag="e")
                          e_dup = sml.tile([128, kmaxb, nh, 2], BF16,
                                           tag="edup")
                          po = pop.tile([128, ch + 8], F32, tag="po")
                          # sub-group size GSZ from cfg; M rotates per group
                          for g0 in range(0, K, GSZ):
                              g1 = min(K, g0 + GSZ)
                              gs = slice(g0, g1)
                              gn_ = g1 - g0
                              gl = slice(0, gn_)
                              M = mp.tile([128, GSZ, ch + 8], BF16, tag="M")
                              nc.vector.tensor_tensor(
                                  al[:, gs, :], Gav[:, gs, 0:nh],
                                  Gav[:, gs, 2 * nh : 3 * nh],
                                  mybir.AluOpType.add,
                              )
                              nc.vector.tensor_tensor(
                                  al[:, gs, :], al[:, gs, :],
                                  Gdav[:, gs, nh : 2 * nh],
                                  mybir.AluOpType.add,
                              )
                              nc.vector.tensor_tensor(
                                  al[:, gs, :], al[:, gs, :],
                                  Gdav[:, gs, 3 * nh : 4 * nh],
                                  mybir.AluOpType.add,
                              )
                              nc.vector.scalar_tensor_tensor(
                                  al[:, gs, :], al[:, gs, :], 0.2,
                                  al[:, gs, :],
                                  mybir.AluOpType.mult, mybir.AluOpType.max,
                              )
                              if agg_cap < 5:
                                  continue
                              nc.scalar.activation(
                                  e_sb[:, gs, :], al[:, gs, :],
                                  mybir.ActivationFunctionType.Exp,
                              )
                              nc.vector.tensor_copy(
                                  M[:, gl, ch : ch + nh], e_sb[:, gs, :]
                              )
                              # scaled messages; pair-duplicated e keeps the
                              # DVE tensor_tensor in 2x_1P mode
                              nc.vector.tensor_copy(
                                  e_dup[:, gs],
                                  e_sb[:, gs].unsqueeze(3).broadcast_to(
                                      [128, gn_, nh, 2]
                                  ),
                              )
                              if FP8 and not MIXED:
                                  # upconvert on ACT, scale in place on DVE
                                  nc.scalar.copy(
                                      M[:, gl, 0:ch], G[:, gs, 0:ch]
                                  )
                                  min_ = M[:, gl, 0:ch]
                              else:
                                  # bf16, or mixed fp8xbf16 DVE read
                                  min_ = G[:, gs, 0:ch]
                              nc.vector.tensor_tensor(
                                  M[:, gl, 0:ch].rearrange(
                                      "p k (h q t) -> p k h q t", h=nh, t=2
                                  ),
                                  min_.rearrange(
                                      "p k (h q t) -> p k h q t", h=nh, t=2
                                  ),
                                  e_dup[:, gs].unsqueeze(3).broadcast_to(
                                      [128, gn_, nh, 64, 2]
                                  ),
                                  mybir.AluOpType.mult,
                              )
                              if agg_cap < 6:
                                  continue
                              # aggregate (last nh cols accumulate the
                              # denominators)
                              for c in range(g0, g1):
                                  first, last = c == 0, c == K - 1
                                  nc.tensor.matmul(
                                      po[:, 0:512], s01_sb[:, c, :],
                                      M[:, c - g0, 0:512],
                                      start=first, stop=last,
                                  )
                                  nc.tensor.matmul(
                                      po[:, 512 : ch + nh], s01_sb[:, c, :],
                                      M[:, c - g0, 512 : ch + nh],
                                      start=first, stop=last,
                                  )
                          if agg_cap < 5 or agg_cap < 6:
                              continue
                          if agg_cap < 61:
                              continue
                          # normalize + head mean + bias + LN + relu
                          den = sml.tile([128, nh], F32, tag="den")
                          nc.vector.tensor_scalar_max(
                              den[:], po[:, ch : ch + nh], 1e-30
                          )
                          rden = sml.tile([128, nh], F32, tag="rden")
                          nc.vector.reciprocal(rden[:], den[:])
                          if agg_cap < 62:
                              continue
                          gn = sml.tile([128, ch], F32, tag="gn")
                          nc.vector.tensor_tensor(
                              gn.rearrange("p (h c) -> p h c", c=128),
                              po[:, 0:ch].rearrange("p (h c) -> p h c", c=128),
                              rden.unsqueeze(2).broadcast_to([128, nh, 128]),
                              mybir.AluOpType.mult,
                          )
                          if agg_cap < 63:
                              continue
                          hm = sml.tile([128, 128], F32, tag="hm")
                          if nh == 6:
                              t2 = sml.tile([128, 384], F32, tag="t2")
                              nc.vector.tensor_tensor(
                                  t2[:], gn[:, 0:384], gn[:, 384:768],
                                  mybir.AluOpType.add,
                              )
                              nc.vector.tensor_tensor(
                                  hm[:], t2[:, 0:128], t2[:, 128:256],
                                  mybir.AluOpType.add,
                              )
                              nc.vector.tensor_tensor(
                                  hm[:], hm[:], t2[:, 256:384],
                                  mybir.AluOpType.add,
                              )
                          else:
                              t2 = sml.tile([128, 256], F32, tag="t2")
                              nc.vector.tensor_tensor(
                                  t2[:], gn[:, 0:256], gn[:, 256:512],
                                  mybir.AluOpType.add,
                              )
                              nc.vector.tensor_tensor(
                                  hm[:], t2[:, 0:128], t2[:, 128:256],
                                  mybir.AluOpType.add,
                              )
                          # hm = hm/nh + bias
                          nc.vector.scalar_tensor_tensor(
                              hm[:], hm[:], 1.0 / nh, cb[:, b_off : b_off + 128],
                              mybir.AluOpType.mult, mybir.AluOpType.add,
                          )
                          if agg_cap < 64:
                              continue
                          # LayerNorm over 128
                          nsum = sml.tile([128, 1], F32, tag="nsum")
                          nc.vector.tensor_reduce(
                              nsum[:], hm[:], mybir.AxisListType.X,
                              mybir.AluOpType.add, negate=True,
                          )
                          nmu = sml.tile([128, 1], F32, tag="nmu")
                          nc.scalar.mul(nmu[:], nsum[:], 1.0 / 128)
                          xc = sml.tile([128, 128], F32, tag="xc")
                          nc.vector.tensor_scalar_add(xc[:], hm[:], nmu[:])
                          if agg_cap < 65:
                              continue
                          sq = sml.tile([128, 128], F32, tag="sq")
                          ss = sml.tile([128, 1], F32, tag="ss")
                          nc.vector.tensor_tensor(
                              sq[:], xc[:], xc[:], mybir.AluOpType.mult
                          )
                          nc.vector.tensor_reduce(
                              ss[:], sq[:], mybir.AxisListType.X,
                              mybir.AluOpType.add,
                          )
                          if agg_cap < 66:
                              continue
                          sd = sml.tile([128, 1], F32, tag="sd")
                          nc.scalar.activation(
                              sd[:], ss[:], mybir.ActivationFunctionType.Sqrt,
                              bias=epsb[:], scale=1.0 / 128,
                          )
                          rstd = sml.tile([128, 1], F32, tag="rstd")
                          nc.vector.reciprocal(rstd[:], sd[:])
                          if agg_cap < 67:
                              continue
                          t3 = sml.tile([128, 128], F32, tag="t3")
                          nc.vector.scalar_tensor_tensor(
                              t3[:], xc[:], rstd[:], cb[:, g_off : g_off + 128],
                              mybir.AluOpType.mult, mybir.AluOpType.mult,
                          )
                          t4 = sml.tile([128, 128], F32, tag="t4")
                          nc.vector.tensor_tensor(
                              t4[:], t3[:], cb[:, be_off : be_off + 128],
                              mybir.AluOpType.add,
                          )
                          xg = sml.tile([128, 128], BF16, tag="xg")
                          nc.vector.tensor_scalar_max(xg[:], t4[:], 0.0)
                          if agg_cap < 68:
                              continue
                          # transpose for the next GEMM
                          pt = ptp.tile([128, 128], BF16, tag="pt")
                          nc.tensor.transpose(pt[:], xg[:], idt[:])
                          nc.vector.tensor_copy(out_T[:, b, :], pt[:])

              # =============== Stage B: GAT1 aggregation ======================
              _mark(nc, "B:gat1-agg")
              x1gat_T = pp.tile([128, ntile, 128], BF16)
              if stage_cap >= 3:
                  gat_agg(ROW1, C1, H1, 0, 128, 256, x1gat_T,
                          H1full[:], h1_stage)

              # =============== Stage C: GCN1 gemm + AG ========================
              _mark(nc, "C:gcn1-gemm+AG")
              if stage_cap >= 4:
                with (
                  tc.tile_pool(name="cps", bufs=2, space="PSUM") as cps,
                  tc.tile_pool(name="cst", bufs=3) as cst,
              ):
                  for t in range(ntile):
                      ps = cps.tile([128, 128], F32, tag="cps")
                      nc.tensor.matmul(ps[:], x1gat_T[:, t, :], wgcn1_sb[:],
                                       start=True, stop=True)
                      yr = cst.tile([128, 128], BF16, tag="yr")
                      nc.scalar.copy(yr[:], ps[:])
                      nc.sync.dma_start(y1_stage[t * 128 : (t + 1) * 128, :], yr[:])
                nc.gpsimd.collective_compute(
                    "AllGather", mybir.AluOpType.bypass, replica_groups=rg,
                    ins=[y1_stage.opt()], outs=[Y1full.opt()],
                )

              # =============== Stage D: GCN1 aggregation ======================
              def gcn_agg(Yfull_ap, ccols, b_off, out_T, out_rows):
                  kmaxb = int(max(Kb))
                  with (
                      tc.tile_pool(name="gy", bufs=3) as gyp,
                      tc.tile_pool(name="sgp", bufs=3) as sgp,
                      tc.tile_pool(name="dsm", bufs=3) as dsm,
                      tc.tile_pool(name="dpo", bufs=2, space="PSUM") as dpo,
                      tc.tile_pool(name="dpt", bufs=2, space="PSUM") as dpt,
                  ):
                      for b in range(nblk):
                          K = int(Kb[b])
                          co = int(chunk_off[b])
                          Gy = gyp.tile([128, kmaxb, 128], BF16, tag="Gy")
                          for c0 in range(0, K, 8):
                              kk = min(8, K - c0)
                              nc.gpsimd.dma_gather(
                                  Gy[:, c0 : c0 + kk, :], Yfull_ap,
                                  idxs[:, (co + c0) * 8 : (co + c0 + kk) * 8],
                                  num_idxs=kk * 128, num_idxs_reg=kk * 128,
                                  elem_size=128, elem_step=128,
                              )
                          sg_sb = sgp.tile([128, kmaxb, 128], BF16, tag="sg")
                          nc.sync.dma_start(
                              sg_sb[:, 0:K, :], sgb[:, co * 128 : (co + K) * 128]
                          )
                          po = dpo.tile([128, ccols], F32, tag="dpo")
                          for c in range(K):
                              nc.tensor.matmul(
                                  po[:], sg_sb[:, c, :], Gy[:, c, 0:ccols],
                                  start=(c == 0), stop=(c == K - 1),
                              )
                          t5 = dsm.tile([128, ccols], F32, tag="t5")
                          nc.vector.tensor_tensor(
                              t5[:], po[:], cb[:, b_off : b_off + ccols],
                              mybir.AluOpType.add,
                          )
                          xg = dsm.tile([128, ccols], BF16, tag="xgc")
                          nc.vector.tensor_scalar_max(xg[:], t5[:], 0.0)
                          if out_rows is not None:
                              nc.vector.tensor_copy(out_rows[:, b, :], xg[:])
                          pt = dpt.tile([128, 128], BF16, tag="dpt")
                          nc.tensor.transpose(
                              pt[0:ccols, 0:128], xg[:, 0:ccols], idt[:]
                          )
                          nc.vector.tensor_copy(
                              out_T[0:ccols, b, :], pt[0:ccols, 0:128]
                          )

              _mark(nc, "D:gcn1-agg")
              if stage_cap >= 5:
                  gcn_agg(Y1full[:], 128, 384, x1gcn_T, None)

              # =============== Stage E: GAT2 gemm + AG ========================
              _mark(nc, "E:gat2-gemm+AG")
              if stage_cap >= 6:
                with (
                  tc.tile_pool(name="eps", bufs=2, space="PSUM") as epsp,
                  tc.tile_pool(name="est", bufs=3) as est,
              ):
                  for t in range(ntile):
                      ps = epsp.tile([128, 520], F32, tag="eps")
                      nc.tensor.matmul(ps[:, 0:512], x1gcn_T[:, t, :],
                                       w2_sb[:, 0:512], start=True, stop=True)
                      nc.tensor.matmul(ps[:, 512:520], x1gcn_T[:, t, :],
                                       w2_sb[:, 512:520], start=True, stop=True)
                      hrow = est.tile([128, ROW2], HD, tag="hrow2")
                      nc.scalar.copy(hrow[:, 0:C2], ps[:, 0:C2])
                      av2 = (hrow[:, C2 : C2 + 32].bitcast(BF16) if FP8
                             else hrow[:, C2 : C2 + 16])
                      nc.vector.tensor_copy(av2[:, 0:8], ps[:, 512:520])
                      nc.vector.tensor_tensor(
                          av2[:, 8:16], ps[:, 512:520],
                          av2[:, 0:8], mybir.AluOpType.subtract,
                      )
                      nc.vector.memset(
                          hrow[:, C2 + (32 if FP8 else 16) : ROW2], 0.0)
                      nc.sync.dma_start(h2_stage[t * 128 : (t + 1) * 128, :], hrow[:])
                nc.gpsimd.collective_compute(
                    "AllGather", mybir.AluOpType.bypass, replica_groups=rg,
                    ins=[h2_stage.opt()], outs=[H2full.opt()],
                )

              # =============== Stage F: GAT2 aggregation ======================
              _mark(nc, "F:gat2-agg")
              x2gat_T = pp.tile([128, ntile, 128], BF16)
              if stage_cap >= 7:
                  gat_agg(ROW2, C2, H2, 512, 640, 768, x2gat_T,
                          H2full[:], h2_stage)

              # =============== Stage G: GCN2 gemm + AG ========================
              _mark(nc, "G:gcn2-gemm+AG")
              if stage_cap >= 8:
                with (
                  tc.tile_pool(name="gps", bufs=2, space="PSUM") as gps,
                  tc.tile_pool(name="gst", bufs=3) as gst,
              ):
                  for t in range(ntile):
                      ps = gps.tile([128, 64], F32, tag="gps")
                      nc.tensor.matmul(ps[:], x2gat_T[:, t, :], wgcn2_sb[:],
                                       start=True, stop=True)
                      yr = gst.tile([128, 128], BF16, tag="yr2")
                      nc.scalar.copy(yr[:, 0:64], ps[:])
                      nc.vector.memset(yr[:, 64:128], 0.0)
                      nc.sync.dma_start(y2_stage[t * 128 : (t + 1) * 128, :], yr[:])
                nc.gpsimd.collective_compute(
                    "AllGather", mybir.AluOpType.bypass, replica_groups=rg,
                    ins=[y2_stage.opt()], outs=[Y2full.opt()],
                )

              # =============== Stage H: GCN2 aggregation ======================
              _mark(nc, "H:gcn2-agg")
              x2gcn_rows = pp.tile([128, ntile, 64], BF16)
              if stage_cap >= 9:
                  gcn_agg(Y2full[:], 64, 896, x2gcn_T, x2gcn_rows)

              # =============== Stage I: fuse + LN3 + classifier ===============
              _mark(nc, "I:fuse+clf")
              if stage_cap >= 10:
                with (
                  tc.tile_pool(name="ips", bufs=2, space="PSUM") as ips,
                  tc.tile_pool(name="ipt", bufs=2, space="PSUM") as ipt,
                  tc.tile_pool(name="icl", bufs=1, space="PSUM") as icl,
                  tc.tile_pool(name="ism", bufs=3) as ism,
                  tc.tile_pool(name="ift", bufs=1) as ift,
              ):
                  fT_all = ift.tile([64, ntile, 128], BF16)
                  for t in range(ntile):
                      pf = ips.tile([128, 64], F32, tag="ip")
                      nc.tensor.matmul(pf[:], x1gcn_T[:, t, :], wf_sb[:, 0, :],
                                       start=True, stop=False)
                      nc.tensor.matmul(pf[:], x2gcn_T[0:64, t, :],
                                       wf_sb[0:64, 1, :], start=False, stop=True)
                      tf = ism.tile([128, 64], F32, tag="tf")
                      nc.vector.tensor_tensor(
                          tf[:], pf[:], cb[:, 1024:1088], mybir.AluOpType.add
                      )
                      nc.vector.tensor_scalar_max(tf[:], tf[:], 0.0)
                      nc.vector.tensor_tensor(
                          tf[:], tf[:], skip_sb[:, t, :], mybir.AluOpType.add
                      )
                      # LN3 over 64
                      nsum = ism.tile([128, 1], F32, tag="insum")
                      nc.vector.tensor_reduce(
                          nsum[:], tf[:], mybir.AxisListType.X,
                          mybir.AluOpType.add, negate=True,
                      )
                      nmu = ism.tile([128, 1], F32, tag="inmu")
                      nc.scalar.mul(nmu[:], nsum[:], 1.0 / 64)
                      xc = ism.tile([128, 64], F32, tag="ixc")
                      nc.vector.tensor_scalar_add(xc[:], tf[:], nmu[:])
                      sq = ism.tile([128, 64], F32, tag="isq")
                      ss = ism.tile([128, 1], F32, tag="iss")
                      nc.vector.tensor_tensor(
                          sq[:], xc[:], xc[:], mybir.AluOpType.mult
                      )
                      nc.vector.tensor_reduce(
                          ss[:], sq[:], mybir.AxisListType.X,
                          mybir.AluOpType.add,
                      )
                      sd = ism.tile([128, 1], F32, tag="isd")
                      nc.scalar.activation(
                          sd[:], ss[:], mybir.ActivationFunctionType.Sqrt,
                          bias=epsb[:], scale=1.0 / 64,
                      )
                      rstd = ism.tile([128, 1], F32, tag="irstd")
                      nc.vector.reciprocal(rstd[:], sd[:])
                      t3 = ism.tile([128, 64], F32, tag="it3")
                      nc.vector.scalar_tensor_tensor(
                          t3[:], xc[:], rstd[:], cb[:, 1088:1152],
                          mybir.AluOpType.mult, mybir.AluOpType.mult,
                      )
                      fin = ism.tile([128, 64], BF16, tag="fin")
                      nc.vector.tensor_tensor(
                          fin[:], t3[:], cb[:, 1152:1216], mybir.AluOpType.add
                      )
                      # stash transposed fin for the batched classifier
                      ptr = ipt.tile([128, 128], BF16, tag="ptr")
                      nc.tensor.transpose(ptr[0:64, 0:128], fin[:, 0:64], idt[:])
                      nc.vector.tensor_copy(fT_all[:, t, :], ptr[0:64, 0:128])
                  # batched classifier in transposed space, 512-col slices:
                  # relu(Wc1^T fT + b) -> relu(Wc2^T . + b) -> Wc3^T . + b
                  NCOLS = ntile * 128
                  fT_f = fT_all.rearrange("p t n -> p (t n)")
                  orow = ism.tile([8, NCOLS], F32, tag="orow")
                  for c0 in range(0, NCOLS, 512):
                      cs = slice(c0, min(NCOLS, c0 + 512))
                      cw = cs.stop - c0
                      p1 = icl.tile([32, 512], F32, tag="p1")
                      nc.tensor.matmul(p1[:, 0:cw], wc1_sb[:], fT_f[:, cs],
                                       start=True, stop=True)
                      h1b = ism.tile([32, 512], BF16, tag="bh1")
                      nc.vector.tensor_scalar(
                          h1b[:, 0:cw], p1[:, 0:cw], cbT_sb[0:32, 0:1], 0.0,
                          mybir.AluOpType.add, mybir.AluOpType.max,
                      )
                      p2 = icl.tile([16, 512], F32, tag="p2")
                      nc.tensor.matmul(p2[:, 0:cw], wc2_sb[:], h1b[:, 0:cw],
                                       start=True, stop=True)
                      h2b = ism.tile([16, 512], BF16, tag="bh2")
                      nc.vector.tensor_scalar(
                          h2b[:, 0:cw], p2[:, 0:cw], cbT_sb[0:16, 1:2], 0.0,
                          mybir.AluOpType.add, mybir.AluOpType.max,
                      )
                      p3 = icl.tile([8, 512], F32, tag="p3")
                      nc.tensor.matmul(p3[:, 0:cw], wc3_sb[:], h2b[:, 0:cw],
                                       start=True, stop=True)
                      nc.vector.tensor_scalar_add(
                          orow[:, cs], p3[:, 0:cw], cbT_sb[0:8, 2:3]
                      )
                  nc.sync.dma_start(out[:, :], orow[:])

            pend = emit_A(0)
            for _r in range(repeat):
                cur = pend
                pend = emit_A(_r + 1) if _r + 1 < repeat else None
                emit_rest(_r, *cur)

            _mark(nc, "Z:end")
            if stage_cap < 10:
                with tc.tile_pool(name="fb", bufs=1) as fb:
                    z = fb.tile([8, n_loc_pad], F32)
                    nc.vector.memset(z[:], 0.0)
                    nc.scalar.dma_start(out[:, :], z[:])

    nc.compile()
    return nc


# ----------------------------------------------------------------------------
# Top-level kernel
# ----------------------------------------------------------------------------

_CACHE = {}
STAGE_MARKS = []


def _mark(nc, label):
    try:
        STAGE_MARKS.append((label, int(nc.next_id())))
    except Exception:
        pass


def prepare(inputs, n_nodes=None, stage_cap=99, agg_cap=99, repeat=1,
            fp8=True, gsz=6, mixed=False, gbufs=2, s01sync=False, pobufs=2):
    """Host prep + (cached) program build. Returns (nc, in_maps, n_loc)."""
    x = np.asarray(inputs["x"], np.float32)
    n = x.shape[0] if n_nodes is None else n_nodes
    f_in = x.shape[1]
    assert n % NCORES == 0
    n_loc = n // NCORES
    n_loc_pad = _rup(n_loc, 128)
    k_pad = _rup(f_in, 128)

    g = prep_graph(inputs["edge_index"], n, n_loc, n_loc_pad)
    w = prep_weights(inputs, k_pad)

    cfg_key = (n_loc_pad, k_pad, w["w1c"], tuple(g["K"]), stage_cap, agg_cap,
               repeat, fp8, gsz, mixed, gbufs, s01sync, pobufs)
    if cfg_key not in _CACHE:
        cfg = {
            "n_loc_pad": n_loc_pad,
            "k_pad": k_pad,
            "w1c": w["w1c"],
            "K": g["K"],
            "totch": g["totch"],
            "nidx": g["nidx"],
            "chunk_off": g["chunk_off"],
            "stage_cap": stage_cap,
            "agg_cap": agg_cap,
            "repeat": repeat,
            "fp8": fp8,
            "gsz": gsz,
            "mixed": mixed,
            "gbufs": gbufs,
            "s01sync": s01sync,
            "pobufs": pobufs,
        }
        _CACHE[cfg_key] = build_nc(cfg)
    nc = _CACHE[cfg_key]

    xp = np.zeros((NCORES * n_loc_pad, k_pad), np.float32)
    for c in range(NCORES):
        xp[c * n_loc_pad : c * n_loc_pad + n_loc, :f_in] = x[
            c * n_loc : (c + 1) * n_loc
        ]
    xpb = xp.astype(NPBF16)

    in_maps = []
    for c in range(NCORES):
        xT_loc = np.ascontiguousarray(
            xpb[c * n_loc_pad : (c + 1) * n_loc_pad].T
        )
        in_maps.append(
            {
                "xT": xT_loc,
                "W1": w["W1"],
                "W2": w["W2"],
                "Wgcn1": w["Wgcn1"],
                "Wgcn2": w["Wgcn2"],
                "Wfuse": w["Wfuse"],
                "Wc1": w["Wc1"],
                "Wc2": w["Wc2"],
                "Wc3": w["Wc3"],
                "cblob": w["cblob"],
                "cbT": w["cbT"],
                "ident": w["ident"],
                "idx16": g["idx16"][c],
                "s01": g["s01"][c],
                "s01T": g["s01T"][c],
                "sg": g["sg"][c],
            }
        )
    return nc, in_maps, n_loc, n_loc_pad


def kernel(**inputs):
    nc, in_maps, n_loc, n_loc_pad = prepare(inputs)
    res = run_bass_kernel_spmd(nc, in_maps, core_ids=list(range(NCORES)))
    n = np.asarray(inputs["x"]).shape[0]
    out = np.zeros((n, 5), np.float32)
    for c in range(NCORES):
        out[c * n_loc : (c + 1) * n_loc] = res.results[c]["out"][:5, :n_loc].T
    return out


if __name__ == "__main__":
    # quick smoke: tiny random problem shaped like the real one
    rng = np.random.default_rng(0)
    N, E, F_IN = 256, 2048, 96
    ip = {
        "x": rng.standard_normal((N, F_IN), dtype=np.float32),
        "edge_index": rng.integers(0, N, (2, E)),
        "W_gat1": rng.standard_normal((F_IN, 768), dtype=np.float32) * 0.05,
        "a_src1": rng.standard_normal((6, 128), dtype=np.float32) * 0.05,
        "a_dst1": rng.standard_normal((6, 128), dtype=np.float32) * 0.05,
        "b_gat1": np.zeros(128, np.float32),
        "W_gcn1": rng.standard_normal((128, 128), dtype=np.float32) * 0.05,
        "b_gcn1": np.zeros(128, np.float32),
        "W_gat2": rng.standard_normal((128, 512), dtype=np.float32) * 0.05,
        "a_src2": rng.standard_normal((4, 128), dtype=np.float32) * 0.05,
        "a_dst2": rng.standard_normal((4, 128), dtype=np.float32) * 0.05,
        "b_gat2": np.zeros(128, np.float32),
        "W_gcn2": rng.standard_normal((128, 64), dtype=np.float32) * 0.05,
        "b_gcn2": np.zeros(64, np.float32),
        "W_skip": rng.standard_normal((F_IN, 64), dtype=np.float32) * 0.05,
        "b_skip": np.zeros(64, np.float32),
        "W_fuse": rng.standard_normal((192, 64), dtype=np.float32) * 0.05,
        "b_fuse": np.zeros(64, np.float32),
        "W_c1": rng.standard_normal((64, 32), dtype=np.float32) * 0.05,
        "b_c1": np.zeros(32, np.float32),
        "W_c2": rng.standard_normal((32, 16), dtype=np.float32) * 0.05,
        "b_c2": np.zeros(16, np.float32),
        "W_c3": rng.standard_normal((16, 5), dtype=np.float32) * 0.05,
        "b_c3": np.zeros(5, np.float32),
        "g1": np.ones(128, np.float32), "be1": np.zeros(128, np.float32),
        "g2": np.ones(128, np.float32), "be2": np.zeros(128, np.float32),
        "g3": np.ones(64, np.float32), "be3": np.zeros(64, np.float32),
    }
    t0 = time.time()
    outv = kernel(**ip)
    print("kernel ran in", time.time() - t0, "shape", outv.shape)
    print(outv[:4])



# revision 19
# speedup vs baseline: 2.2705x; 1.4508x over previous
"""Trainium2 Bass kernel for nn_AdvancedCardiomyocyteGNN (GAT/GCN message passing).

Strategy (8 NeuronCores, SPMD single NEFF):
  - Nodes sharded across cores (1250 -> padded 1280 per core).
  - Node-wise GEMMs computed on the owning core; per-edge alpha projections
    (h . a_src / h . a_dst) are folded into the main GEMM weights on the host.
  - AllGather replicates the transformed node features (bf16) to all cores.
  - Edges partitioned by dst, sorted, grouped per 128-dst block; source rows
    are fetched with dma_gather (128 edges per chunk land on 128 partitions);
    segment softmax + weighted segment sum are computed as one-hot matmuls
    (S^T @ M accumulated in PSUM per dst block).
  - Graph-structure-dependent one-hot/scatter matrices and index tables are
    precomputed on the host (pure preprocessing of the integer edge list).
"""

import sys
import time

sys.path.insert(0, "/opt/trn_rl_repo")

import numpy as np
import ml_dtypes

import concourse.bass as bass
import concourse.tile as tile
from concourse import bacc, mybir
from concourse.bass_utils import run_bass_kernel_spmd

F32 = mybir.dt.float32
BF16 = mybir.dt.bfloat16
F8 = mybir.dt.float8e4
I16 = mybir.dt.int16
NPBF16 = ml_dtypes.bfloat16

NCORES = 8


def _rup(x, m):
    return (x + m - 1) // m * m


# ----------------------------------------------------------------------------
# Host-side graph preprocessing
# ----------------------------------------------------------------------------

def prep_graph(edge_index, n_nodes, n_loc, n_loc_pad, heads_dummy=None):
    """Partition edges (with self loops) by dst across cores, sort by dst,
    group per 128-dst block, pad each block to a per-block common chunk count.

    Returns dict with per-core index/scatter data and layout constants."""
    src = np.asarray(edge_index[0], dtype=np.int64)
    dst = np.asarray(edge_index[1], dtype=np.int64)
    loop = np.arange(n_nodes, dtype=np.int64)
    src = np.concatenate([src, loop])
    dst = np.concatenate([dst, loop])

    # gcn normalization (reference: deg over dst including self loops)
    deg = np.bincount(dst, minlength=n_nodes).astype(np.float64)
    dinv = np.where(deg > 0, deg ** -0.5, 0.0)
    ce_all = (dinv[src] * dinv[dst]).astype(np.float32)

    # padded node ids
    def pad_id(n):
        return (n // n_loc) * n_loc_pad + (n % n_loc)

    srcp = pad_id(src)
    dstp = pad_id(dst)

    core_of = dst // n_loc
    nblk = n_loc_pad // 128

    # per (core, blk) edge lists
    per_core = []
    for c in range(NCORES):
        m = core_of == c
        s, d, ce = srcp[m], dstp[m], ce_all[m]
        dloc = d - c * n_loc_pad
        order = np.argsort(dloc, kind="stable")
        s, dloc, ce = s[order], dloc[order], ce[order]
        blk = dloc // 128
        per_core.append((s, dloc, ce, blk))

    # per-block chunk count, common across cores
    K = np.zeros(nblk, dtype=np.int64)
    for c in range(NCORES):
        _, _, _, blk = per_core[c]
        cnt = np.bincount(blk, minlength=nblk)
        K = np.maximum(K, (cnt + 127) // 128)
    K = np.maximum(K, 1).astype(int)
    totch = int(K.sum())
    nidx = totch * 128

    idx16 = np.zeros((NCORES, 128, nidx // 16), dtype=np.int16)
    s01 = np.zeros((NCORES, 128, totch * 128), dtype=NPBF16)
    s01T = np.zeros((NCORES, 128, totch * 128), dtype=NPBF16)
    sg = np.zeros((NCORES, 128, totch * 128), dtype=NPBF16)

    chunk_off = np.concatenate([[0], np.cumsum(K)])  # chunk offset per block

    for c in range(NCORES):
        s, dloc, ce, blk = per_core[c]
        idx_flat = np.zeros(nidx, dtype=np.int16)
        for b in range(nblk):
            m = blk == b
            sb_, db_, cb_ = s[m], dloc[m] - b * 128, ce[m]
            ne = len(sb_)
            base = chunk_off[b] * 128  # edge slot offset
            idx_flat[base : base + ne] = sb_.astype(np.int16)
            ch = base // 128 + np.arange(ne) // 128  # global chunk id
            e_in = np.arange(ne) % 128
            s01[c, e_in, ch * 128 + db_] = NPBF16(1.0)
            s01T[c, db_, ch * 128 + e_in] = NPBF16(1.0)
            sg[c, e_in, ch * 128 + db_] = cb_.astype(NPBF16)
        idx16[c] = np.tile(idx_flat.reshape(-1, 16).T, (8, 1))

    return {
        "K": K,
        "totch": totch,
        "nidx": nidx,
        "chunk_off": chunk_off,
        "idx16": idx16,
        "s01": s01,
        "s01T": s01T,
        "sg": sg,
    }


def prep_weights(ip, k_pad):
    """Fold/concat/cast weights on the host. Returns dict of shared arrays."""
    f32 = np.float32
    w_gat1 = np.asarray(ip["W_gat1"], f32)  # [F_IN, 768]
    a_src1 = np.asarray(ip["a_src1"], f32)  # [6, 128]
    a_dst1 = np.asarray(ip["a_dst1"], f32)
    w_skip = np.asarray(ip["W_skip"], f32)  # [F_IN, 64]
    f_in = w_gat1.shape[0]
    h1 = a_src1.shape[0]
    c1 = a_src1.shape[1]
    ws1 = np.einsum("khc,hc->kh", w_gat1.reshape(f_in, h1, c1), a_src1)
    wd1 = np.einsum("khc,hc->kh", w_gat1.reshape(f_in, h1, c1), a_dst1)
    w1 = np.concatenate([w_gat1, w_skip, ws1, wd1], axis=1)  # [F_IN, 844]
    w1c = _rup(w1.shape[1], 16)
    w1p = np.zeros((k_pad, w1c), NPBF16)
    w1p[:f_in, : w1.shape[1]] = w1.astype(NPBF16)

    w_gat2 = np.asarray(ip["W_gat2"], f32)  # [128, 512]
    a_src2 = np.asarray(ip["a_src2"], f32)  # [4, 128]
    a_dst2 = np.asarray(ip["a_dst2"], f32)
    h2 = a_src2.shape[0]
    ws2 = np.einsum("khc,hc->kh", w_gat2.reshape(128, h2, c1), a_src2)
    wd2 = np.einsum("khc,hc->kh", w_gat2.reshape(128, h2, c1), a_dst2)
    w2 = np.concatenate([w_gat2, ws2, wd2], axis=1)  # [128, 520]
    w2p = w2.astype(NPBF16)

    def rep(v, cols=None):
        v = np.asarray(v, f32).reshape(-1)
        if cols is not None:
            vv = np.zeros(cols, f32)
            vv[: len(v)] = v
            v = vv
        return np.tile(v[None, :], (128, 1)).astype(f32)

    cblob = np.concatenate(
        [
            rep(ip["b_gat1"]),   # 0:128
            rep(ip["g1"]),       # 128:256
            rep(ip["be1"]),      # 256:384
            rep(ip["b_gcn1"]),   # 384:512
            rep(ip["b_gat2"]),   # 512:640
            rep(ip["g2"]),       # 640:768
            rep(ip["be2"]),      # 768:896
            rep(ip["b_gcn2"], 64),   # 896:960
            rep(ip["b_skip"], 64),   # 960:1024
            rep(ip["b_fuse"], 64),   # 1024:1088
            rep(ip["g3"], 64),       # 1088:1152
            rep(ip["be3"], 64),      # 1152:1216
            rep(ip["b_c1"], 32),     # 1216:1248
            rep(ip["b_c2"], 16),     # 1248:1264
            rep(ip["b_c3"], 8),      # 1264:1272
        ],
        axis=1,
    )

    wf = np.asarray(ip["W_fuse"], f32)  # [192, 64]
    wc1 = np.asarray(ip["W_c1"], f32)  # [64, 32]
    wc2 = np.asarray(ip["W_c2"], f32)  # [32, 16]
    wc3 = np.asarray(ip["W_c3"], f32)  # [16, 5]
    wc3p = np.zeros((wc3.shape[0], 8), np.float32)
    wc3p[:, : wc3.shape[1]] = wc3

    cbT = np.zeros((128, 4), f32)
    cbT[:32, 0] = np.asarray(ip["b_c1"], f32)
    cbT[:16, 1] = np.asarray(ip["b_c2"], f32)
    cbT[:5, 2] = np.asarray(ip["b_c3"], f32)

    return {
        "W1": w1p,
        "W2": w2p,
        "Wgcn1": np.asarray(ip["W_gcn1"], f32).astype(NPBF16),
        "Wgcn2": np.asarray(ip["W_gcn2"], f32).astype(NPBF16),
        "Wfuse": wf.astype(NPBF16),
        "Wc1": wc1.astype(NPBF16),
        "Wc2": wc2.astype(NPBF16),
        "Wc3": wc3p.astype(NPBF16),
        "cblob": cblob,
        "cbT": cbT,
        "ident": np.eye(128, dtype=NPBF16),
        "w1c": w1c,
    }


# ----------------------------------------------------------------------------
# Bass program builder
# ----------------------------------------------------------------------------

def build_nc(cfg):
    """cfg: dict with n_loc_pad, k_pad (F_IN padded), K (list per block),
    totch, nidx, w1c, h1=6, h2=4."""
    n_loc_pad = cfg["n_loc_pad"]
    k_pad = cfg["k_pad"]
    Kb = cfg["K"]
    totch = cfg["totch"]
    nidx = cfg["nidx"]
    w1c = cfg["w1c"]
    chunk_off = cfg["chunk_off"]
    nblk = n_loc_pad // 128
    ntile = nblk
    kch = k_pad // 128
    NP = NCORES * n_loc_pad
    H1, H2 = 6, 4
    FP8 = bool(cfg.get("fp8", False))
    GSZ = int(cfg.get("gsz", 6))
    MIXED = bool(cfg.get("mixed", False))
    GB = int(cfg.get("gbufs", 2))
    S01SYNC = bool(cfg.get("s01sync", False))
    POB = int(cfg.get("pobufs", 2))
    HD = F8 if FP8 else BF16
    if FP8:
        # fp8 rows: [h fp8 | alpha hi/lo as raw bf16 bytes | pad to 256B]
        ROW1 = _rup(H1 * 128 + 2 * 24, 256)   # 1024
        ROW2 = _rup(H2 * 128 + 2 * 16, 256)   # 768
    else:
        ROW1 = _rup(H1 * 128 + 24, 128)   # h(768)+asrc hi/lo+adst hi/lo
        ROW2 = _rup(H2 * 128 + 16, 128)
    C1 = H1 * 128
    C2 = H2 * 128
    AS1 = C1 + 24   # alpha block end (asrc hi/lo + adst hi/lo)
    AS2 = C2 + 16
    EPS = 1e-5

    stage_cap = cfg.get("stage_cap", 99)
    agg_cap = cfg.get("agg_cap", 99)
    repeat = cfg.get("repeat", 1)
    STAGE_MARKS.clear()
    nc = bacc.Bacc("TRN2", target_bir_lowering=False, debug=False,
                   num_devices=NCORES, num_swdge_queues=2)

    def din(name, shape, dt):
        return nc.dram_tensor(name, shape, dt, kind="ExternalInput").ap()

    xT = din("xT", [k_pad, n_loc_pad], BF16)
    W1 = din("W1", [k_pad, w1c], BF16)
    W2 = din("W2", [128, 520], BF16)
    Wgcn1 = din("Wgcn1", [128, 128], BF16)
    Wgcn2 = din("Wgcn2", [128, 64], BF16)
    Wfuse = din("Wfuse", [192, 64], BF16)
    Wc1 = din("Wc1", [64, 32], BF16)
    Wc2 = din("Wc2", [32, 16], BF16)
    Wc3 = din("Wc3", [16, 8], BF16)
    cblob = din("cblob", [128, 1272], F32)
    ident = din("ident", [128, 128], BF16)
    idx16 = din("idx16", [128, nidx // 16], I16)
    s01b = din("s01", [128, totch * 128], BF16)
    s01Tb = din("s01T", [128, totch * 128], BF16)
    sgb = din("sg", [128, totch * 128], BF16)

    cbT = din("cbT", [128, 4], F32)
    out = nc.dram_tensor("out", [8, n_loc_pad], F32, kind="ExternalOutput").ap()

    rg = [list(range(NCORES))]

    _qctr = [0]

    def _qn():
        _qctr[0] ^= 1
        return _qctr[0]

    with tile.TileContext(nc) as tc:
        with (
            tc.tile_pool(name="const", bufs=1) as cpool,
            tc.tile_pool(name="persist", bufs=2) as pp,
            tc.tile_pool(name="xtp", bufs=3) as axp,
            tc.tile_pool(name="dram", bufs=1, space="DRAM") as dpool,
        ):
            # ---- constants / persistent tiles ----
            cb = cpool.tile([128, 1272], F32)
            nc.scalar.dma_start(cb[:], cblob[:])
            idt = cpool.tile([128, 128], BF16)
            nc.scalar.dma_start(idt[:], ident[:])
            idxs = cpool.tile([128, nidx // 16], I16)
            nc.scalar.dma_start(idxs[:], idx16[:])
            wgcn1_sb = cpool.tile([128, 128], BF16)
            nc.scalar.dma_start(wgcn1_sb[:], Wgcn1[:])
            wgcn2_sb = cpool.tile([128, 64], BF16)
            nc.scalar.dma_start(wgcn2_sb[:], Wgcn2[:])
            w2_sb = cpool.tile([128, 520], BF16)
            nc.scalar.dma_start(w2_sb[:], W2[:])
            wf_sb = cpool.tile([128, 2, 64], BF16)
            nc.scalar.dma_start(wf_sb[:, 0, :], Wfuse[0:128, :])
            nc.scalar.dma_start(wf_sb[:64, 1, :], Wfuse[128:192, :])
            wc1_sb = cpool.tile([64, 32], BF16)
            nc.scalar.dma_start(wc1_sb[:], Wc1[:])
            wc2_sb = cpool.tile([32, 16], BF16)
            nc.scalar.dma_start(wc2_sb[:], Wc2[:])
            wc3_sb = cpool.tile([16, 8], BF16)
            nc.scalar.dma_start(wc3_sb[:], Wc3[:])
            epsb = cpool.tile([128, 1], F32)
            nc.vector.memset(epsb[:], EPS)
            cbT_sb = cpool.tile([128, 4], F32)
            nc.scalar.dma_start(cbT_sb[:], cbT[:])
            # weights for GEMM1 live in SBUF across reps
            w1_sb = cpool.tile([128, kch, w1c], BF16)
            W1_r = W1.rearrange("(c p) n -> p c n", p=128)
            for c in range(kch):
                nc.scalar.dma_start(w1_sb[:, c, :], W1_r[:, c, :])

            # =============== Stage A: GEMM1 (x @ [Wgat1|Wskip|Ws|Wd]) =======
            # Software-pipelined emission: stage A (+ its AllGather) of rep
            # r+1 is emitted BEFORE stages B..I of rep r so the per-engine
            # in-order queues can overlap the next rep's GEMM and AG wire
            # time with the current rep's aggregation work.
            def emit_A(_rep):
              _mark(nc, f"A:gemm1 r{_rep}")
              h1_stage = dpool.tile([n_loc_pad, ROW1], HD, tag=f"h1s{_rep}")
              H1full = dpool.tile([NP, ROW1], HD, addr_space="Shared",
                                  tag=f"H1f{_rep}")
              skip_sb = pp.tile([128, ntile, 64], F32, tag="skip")
              with (
                  tc.tile_pool(name="apsum", bufs=1, space="PSUM") as apsum,
                  tc.tile_pool(name="astage", bufs=3) as astage,
              ):
                  xT_r = xT.rearrange("(c p) n -> p c n", p=128)
                  for t in range(ntile):
                      ns = t * 128
                      xt_sb = axp.tile([128, kch, 128], BF16, tag="xt")
                      nc.sync.dma_start(xt_sb[:], xT_r[:, :, ns : ns + 128])
                      ps = apsum.tile([128, w1c], F32, tag="ps")
                      for c in range(kch):
                          lhsT = xt_sb[:, c, :]
                          nc.tensor.matmul(
                              ps[:, 0:512], lhsT, w1_sb[:, c, 0:512],
                              start=(c == 0), stop=(c == kch - 1),
                          )
                          nc.tensor.matmul(
                              ps[:, 512:w1c], lhsT, w1_sb[:, c, 512:w1c],
                              start=(c == 0), stop=(c == kch - 1),
                          )
                      hrow = astage.tile([128, ROW1], HD, tag="hrow")
                      nc.scalar.copy(hrow[:, 0:C1], ps[:, 0:C1])
                      av = (hrow[:, C1 : C1 + 48].bitcast(BF16) if FP8
                            else hrow[:, C1 : C1 + 24])
                      nc.vector.tensor_copy(
                          av[:, 0:12], ps[:, C1 + 64 : C1 + 76]
                      )
                      nc.vector.tensor_tensor(
                          av[:, 12:24],
                          ps[:, C1 + 64 : C1 + 76],
                          av[:, 0:12], mybir.AluOpType.subtract,
                      )
                      nc.vector.memset(
                          hrow[:, C1 + (48 if FP8 else 24) : ROW1], 0.0)
                      nc.sync.dma_start(h1_stage[ns : ns + 128, :], hrow[:])
                      # skip = relu(x@Wskip + b_skip)
                      tsk = astage.tile([128, 64], F32, tag="tsk")
                      nc.vector.tensor_tensor(
                          tsk[:], ps[:, C1 : C1 + 64], cb[:, 960:1024],
                          mybir.AluOpType.add,
                      )
                      nc.vector.tensor_scalar_max(skip_sb[:, t, :], tsk[:], 0.0)

              _mark(nc, "A2:AG-H1")
              if stage_cap >= 2:
                  nc.gpsimd.collective_compute(
                      "AllGather", mybir.AluOpType.bypass, replica_groups=rg,
                      ins=[h1_stage.opt()], outs=[H1full.opt()],
                  )
              return h1_stage, H1full, skip_sb

            if True:
              # =============== helper: GAT aggregation stage ==================
              def gat_agg(row, ch, nh, b_off, g_off, be_off,
                          out_T, src_full_ap, stage_ap):
                  """Per dst-block: gather rows, softmax-weighted segment sum,
                  head mean + bias + LN + relu; writes [nodes,128] bf16 blocks
                  transposed into out_T."""
                  kmaxb = int(max(Kb))
                  with (
                      tc.tile_pool(name=f"g{nh}", bufs=GB) as gp,
                      tc.tile_pool(name=f"gd{nh}", bufs=3) as gdp,
                      tc.tile_pool(name=f"m{nh}", bufs=2) as mp,
                      tc.tile_pool(name=f"s{nh}", bufs=2) as sp,
                      tc.tile_pool(name=f"sml{nh}", bufs=2) as sml,
                      tc.tile_pool(name=f"po{nh}", bufs=POB, space="PSUM") as pop,
                      tc.tile_pool(name=f"pt{nh}", bufs=2, space="PSUM") as ptp,
                      tc.tile_pool(name=f"pa{nh}", bufs=2, space="PSUM") as pap,
                  ):
                      for b in range(nblk):
                          K = int(Kb[b])
                          co = int(chunk_off[b])
                          G = gp.tile([128, kmaxb, row], HD, tag="G")
                          for c0 in range(0, K, 8):
                              kk = min(8, K - c0)
                              nc.gpsimd.dma_gather(
                                  G[:, c0 : c0 + kk, :], src_full_ap,
                                  idxs[:, (co + c0) * 8 : (co + c0 + kk) * 8],
                                  num_idxs=kk * 128, num_idxs_reg=kk * 128,
                                  elem_size=row, elem_step=row,
                                  queue_num=_qn(),
                              )
                          # dst-alpha block: the dst rows of block b are the
                          # core's OWN stage rows (local, pre-collective);
                          # broadcast dst alpha to edge slots via per-chunk
                          # matmul with the transposed one-hot s01T.
                          acols = 8 * nh if FP8 else 4 * nh
                          adst = gdp.tile([128, acols], HD, tag="adst")
                          nc.sync.dma_start(
                              adst[:],
                              stage_ap[b * 128 : (b + 1) * 128, ch : ch + acols],
                          )
                          adst_bf = (adst.bitcast(BF16) if FP8 else adst)
                          s01T_sb = gdp.tile([128, kmaxb, 128], BF16, tag="s01T")
                          nc.sync.dma_start(
                              s01T_sb[:, 0:K, :],
                              s01Tb[:, co * 128 : (co + K) * 128],
                          )
                          ald = pap.tile([128, kmaxb, 4 * nh], F32, tag="ald")
                          for c in range(K):
                              nc.tensor.matmul(
                                  ald[:, c, :], s01T_sb[:, c, :],
                                  adst_bf[:, 0 : 4 * nh],
                                  start=True, stop=True,
                              )
                          # bf16 views of the alpha cols (raw bytes when fp8)
                          Gav = (G[:, :, ch : ch + 8 * nh].bitcast(BF16)
                                 if FP8 else G[:, :, ch : ch + 4 * nh])
                          Gdav = ald
                          if agg_cap < 2:
                              continue
                          s01_sb = sp.tile([128, kmaxb, 128], BF16, tag="s01")
                          nc.sync.dma_start(
                              s01_sb[:, 0:K, :], s01b[:, co * 128 : (co + K) * 128]
                          )
                          if agg_cap < 4:
                              continue
                          # alpha = lrelu(asrc_src + adst_dst); e = exp(alpha)
                          # processed in sub-groups of GSZ chunks so PE can
                          # start aggregating while DVE scales later groups
                          al = sml.tile([128, kmaxb, nh], F32, tag="al")
                          e_sb = sml.tile([128, kmaxb, nh], F32, tag="e")
                          e_dup = sml.tile([128, kmaxb, nh, 2], BF16,
                                           tag="edup")
                          po = pop.tile([128, ch + 8], F32, tag="po")
                          # sub-group size GSZ from cfg; M rotates per group
                          for g0 in range(0, K, GSZ):
                              g1 = min(K, g0 + GSZ)
                              gs = slice(g0, g1)
                              gn_ = g1 - g0
                              gl = slice(0, gn_)
                              M = mp.tile([128, GSZ, ch + 8], BF16, tag="M")
                              nc.vector.tensor_tensor(
                                  al[:, gs, :], Gav[:, gs, 0:nh],
                                  Gav[:, gs, 2 * nh : 3 * nh],
                                  mybir.AluOpType.add,
                              )
                              nc.vector.tensor_tensor(
                                  al[:, gs, :], al[:, gs, :],
                                  Gdav[:, gs, nh : 2 * nh],
                                  mybir.AluOpType.add,
                              )
                              nc.vector.tensor_tensor(
                                  al[:, gs, :], al[:, gs, :],
                                  Gdav[:, gs, 3 * nh : 4 * nh],
                                  mybir.AluOpType.add,
                              )
                              nc.vector.scalar_tensor_tensor(
                                  al[:, gs, :], al[:, gs, :], 0.2,
                                  al[:, gs, :],
                                  mybir.AluOpType.mult, mybir.AluOpType.max,
                              )
                              if agg_cap < 5:
                                  continue
                              nc.scalar.activation(
                                  e_sb[:, gs, :], al[:, gs, :],
                                  mybir.ActivationFunctionType.Exp,
                              )
                              nc.vector.tensor_copy(
                                  M[:, gl, ch : ch + nh], e_sb[:, gs, :]
                              )
                              # scaled messages; pair-duplicated e keeps the
                              # DVE tensor_tensor in 2x_1P mode
                              nc.vector.tensor_copy(
                                  e_dup[:, gs],
                                  e_sb[:, gs].unsqueeze(3).broadcast_to(
                                      [128, gn_, nh, 2]
                                  ),
                              )
                              if FP8 and not MIXED:
                                  # upconvert on ACT, scale in place on DVE
                                  nc.scalar.copy(
                                      M[:, gl, 0:ch], G[:, gs, 0:ch]
                                  )
                                  min_ = M[:, gl, 0:ch]
                              else:
                                  # bf16, or mixed fp8xbf16 DVE read
                                  min_ = G[:, gs, 0:ch]
                              nc.vector.tensor_tensor(
                                  M[:, gl, 0:ch].rearrange(
                                      "p k (h q t) -> p k h q t", h=nh, t=2
                                  ),
                                  min_.rearrange(
                                      "p k (h q t) -> p k h q t", h=nh, t=2
                                  ),
                                  e_dup[:, gs].unsqueeze(3).broadcast_to(
                                      [128, gn_, nh, 64, 2]
                                  ),
                                  mybir.AluOpType.mult,
                              )
                              if agg_cap < 6:
                                  continue
                              # aggregate (last nh cols accumulate the
                              # denominators)
                              for c in range(g0, g1):
                                  first, last = c == 0, c == K - 1
                                  nc.tensor.matmul(
                                      po[:, 0:512], s01_sb[:, c, :],
                                      M[:, c - g0, 0:512],
                                      start=first, stop=last,
                                  )
                                  nc.tensor.matmul(
                                      po[:, 512 : ch + nh], s01_sb[:, c, :],
                                      M[:, c - g0, 512 : ch + nh],
                                      start=first, stop=last,
                                  )
                          if agg_cap < 5 or agg_cap < 6:
                              continue
                          if agg_cap < 61:
                              continue
                          # normalize + head mean + bias + LN + relu
                          den = sml.tile([128, nh], F32, tag="den")
                          nc.vector.tensor_scalar_max(
                              den[:], po[:, ch : ch + nh], 1e-30
                          )
                          rden = sml.tile([128, nh], F32, tag="rden")
                          nc.vector.reciprocal(rden[:], den[:])
                          if agg_cap < 62:
                              continue
                          gn = sml.tile([128, ch], F32, tag="gn")
                          nc.vector.tensor_tensor(
                              gn.rearrange("p (h c) -> p h c", c=128),
                              po[:, 0:ch].rearrange("p (h c) -> p h c", c=128),
                              rden.unsqueeze(2).broadcast_to([128, nh, 128]),
                              mybir.AluOpType.mult,
                          )
                          if agg_cap < 63:
                              continue
                          hm = sml.tile([128, 128], F32, tag="hm")
                          if nh == 6:
                              t2 = sml.tile([128, 384], F32, tag="t2")
                              nc.vector.tensor_tensor(
                                  t2[:], gn[:, 0:384], gn[:, 384:768],
                                  mybir.AluOpType.add,
                              )
                              nc.vector.tensor_tensor(
                                  hm[:], t2[:, 0:128], t2[:, 128:256],
                                  mybir.AluOpType.add,
                              )
                              nc.vector.tensor_tensor(
                                  hm[:], hm[:], t2[:, 256:384],
                                  mybir.AluOpType.add,
                              )
                          else:
                              t2 = sml.tile([128, 256], F32, tag="t2")
                              nc.vector.tensor_tensor(
                                  t2[:], gn[:, 0:256], gn[:, 256:512],
                                  mybir.AluOpType.add,
                              )
                              nc.vector.tensor_tensor(
                                  hm[:], t2[:, 0:128], t2[:, 128:256],
                                  mybir.AluOpType.add,
                              )
                          # hm = hm/nh + bias
                          nc.vector.scalar_tensor_tensor(
                              hm[:], hm[:], 1.0 / nh, cb[:, b_off : b_off + 128],
                              mybir.AluOpType.mult, mybir.AluOpType.add,
                          )
                          if agg_cap < 64:
                              continue
                          # LayerNorm over 128
                          nsum = sml.tile([128, 1], F32, tag="nsum")
                          nc.vector.tensor_reduce(
                              nsum[:], hm[:], mybir.AxisListType.X,
                              mybir.AluOpType.add, negate=True,
                          )
                          nmu = sml.tile([128, 1], F32, tag="nmu")
                          nc.scalar.mul(nmu[:], nsum[:], 1.0 / 128)
                          xc = sml.tile([128, 128], F32, tag="xc")
                          nc.vector.tensor_scalar_add(xc[:], hm[:], nmu[:])
                          if agg_cap < 65:
                              continue
                          sq = sml.tile([128, 128], F32, tag="sq")
                          ss = sml.tile([128, 1], F32, tag="ss")
                          nc.vector.tensor_tensor(
                              sq[:], xc[:], xc[:], mybir.AluOpType.mult
                          )
                          nc.vector.tensor_reduce(
                              ss[:], sq[:], mybir.AxisListType.X,
                              mybir.AluOpType.add,
                          )
                          if agg_cap < 66:
                              continue
                          sd = sml.tile([128, 1], F32, tag="sd")
                          nc.scalar.activation(
                              sd[:], ss[:], mybir.ActivationFunctionType.Sqrt,
                              bias=epsb[:], scale=1.0 / 128,
                          )
                          rstd = sml.tile([128, 1], F32, tag="rstd")
                          nc.vector.reciprocal(rstd[:], sd[:])
                          if agg_cap < 67:
                              continue
                          t3 = sml.tile([128, 128], F32, tag="t3")
                          nc.vector.scalar_tensor_tensor(
                              t3[:], xc[:], rstd[:], cb[:, g_off : g_off + 128],
                              mybir.AluOpType.mult, mybir.AluOpType.mult,
                          )
                          t4 = sml.tile([128, 128], F32, tag="t4")
                          nc.vector.tensor_tensor(
                              t4[:], t3[:], cb[:, be_off : be_off + 128],
                              mybir.AluOpType.add,
                          )
                          xg = sml.tile([128, 128], BF16, tag="xg")
                          nc.vector.tensor_scalar_max(xg[:], t4[:], 0.0)
                          if agg_cap < 68:
                              continue
                          # transpose for the next GEMM
                          pt = ptp.tile([128, 128], BF16, tag="pt")
                          nc.tensor.transpose(pt[:], xg[:], idt[:])
                          nc.vector.tensor_copy(out_T[:, b, :], pt[:])

              # =============== Stage B: GAT1 aggregation ======================
              _mark(nc, "B:gat1-agg")
              x1gat_T = pp.tile([128, ntile, 128], BF16)
              if stage_cap >= 3:
                  gat_agg(ROW1, C1, H1, 0, 128, 256, x1gat_T,
                          H1full[:], h1_stage)

              # =============== Stage C: GCN1 gemm + AG ========================
              _mark(nc, "C:gcn1-gemm+AG")
              if stage_cap >= 4:
                with (
                  tc.tile_pool(name="cps", bufs=2, space="PSUM") as cps,
                  tc.tile_pool(name="cst", bufs=3) as cst,
              ):
                  for t in range(ntile):
                      ps = cps.tile([128, 128], F32, tag="cps")
                      nc.tensor.matmul(ps[:], x1gat_T[:, t, :], wgcn1_sb[:],
                                       start=True, stop=True)
                      yr = cst.tile([128, 128], BF16, tag="yr")
                      nc.scalar.copy(yr[:], ps[:])
                      nc.sync.dma_start(y1_stage[t * 128 : (t + 1) * 128, :], yr[:])
                nc.gpsimd.collective_compute(
                    "AllGather", mybir.AluOpType.bypass, replica_groups=rg,
                    ins=[y1_stage.opt()], outs=[Y1full.opt()],
                )

              # =============== Stage D: GCN1 aggregation ======================
              def gcn_agg(Yfull_ap, ccols, b_off, out_T, out_rows):
                  kmaxb = int(max(Kb))
                  with (
                      tc.tile_pool(name="gy", bufs=3) as gyp,
                      tc.tile_pool(name="sgp", bufs=3) as sgp,
                      tc.tile_pool(name="dsm", bufs=3) as dsm,
                      tc.tile_pool(name="dpo", bufs=2, space="PSUM") as dpo,
                      tc.tile_pool(name="dpt", bufs=2, space="PSUM") as dpt,
                  ):
                      for b in range(nblk):
                          K = int(Kb[b])
                          co = int(chunk_off[b])
                          Gy = gyp.tile([128, kmaxb, 128], BF16, tag="Gy")
                          for c0 in range(0, K, 8):
                              kk = min(8, K - c0)
                              nc.gpsimd.dma_gather(
                                  Gy[:, c0 : c0 + kk, :], Yfull_ap,
                                  idxs[:, (co + c0) * 8 : (co + c0 + kk) * 8],
                                  num_idxs=kk * 128, num_idxs_reg=kk * 128,
                                  elem_size=128, elem_step=128,
                                  queue_num=_qn(),
                              )
                          sg_sb = sgp.tile([128, kmaxb, 128], BF16, tag="sg")
                          nc.sync.dma_start(
                              sg_sb[:, 0:K, :], sgb[:, co * 128 : (co + K) * 128]
                          )
                          po = dpo.tile([128, ccols], F32, tag="dpo")
                          for c in range(K):
                              nc.tensor.matmul(
                                  po[:], sg_sb[:, c, :], Gy[:, c, 0:ccols],
                                  start=(c == 0), stop=(c == K - 1),
                              )
                          t5 = dsm.tile([128, ccols], F32, tag="t5")
                          nc.vector.tensor_tensor(
                              t5[:], po[:], cb[:, b_off : b_off + ccols],
                              mybir.AluOpType.add,
                          )
                          xg = dsm.tile([128, ccols], BF16, tag="xgc")
                          nc.vector.tensor_scalar_max(xg[:], t5[:], 0.0)
                          if out_rows is not None:
                              nc.vector.tensor_copy(out_rows[:, b, :], xg[:])
                          pt = dpt.tile([128, 128], BF16, tag="dpt")
                          nc.tensor.transpose(
                              pt[0:ccols, 0:128], xg[:, 0:ccols], idt[:]
                          )
                          nc.vector.tensor_copy(
                              out_T[0:ccols, b, :], pt[0:ccols, 0:128]
                          )

              _mark(nc, "D:gcn1-agg")
              if stage_cap >= 5:
                  gcn_agg(Y1full[:], 128, 384, x1gcn_T, None)

              # =============== Stage E: GAT2 gemm + AG ========================
              _mark(nc, "E:gat2-gemm+AG")
              if stage_cap >= 6:
                with (
                  tc.tile_pool(name="eps", bufs=2, space="PSUM") as epsp,
                  tc.tile_pool(name="est", bufs=3) as est,
              ):
                  for t in range(ntile):
                      ps = epsp.tile([128, 520], F32, tag="eps")
                      nc.tensor.matmul(ps[:, 0:512], x1gcn_T[:, t, :],
                                       w2_sb[:, 0:512], start=True, stop=True)
                      nc.tensor.matmul(ps[:, 512:520], x1gcn_T[:, t, :],
                                       w2_sb[:, 512:520], start=True, stop=True)
                      hrow = est.tile([128, ROW2], HD, tag="hrow2")
                      nc.scalar.copy(hrow[:, 0:C2], ps[:, 0:C2])
                      av2 = (hrow[:, C2 : C2 + 32].bitcast(BF16) if FP8
                             else hrow[:, C2 : C2 + 16])
                      nc.vector.tensor_copy(av2[:, 0:8], ps[:, 512:520])
                      nc.vector.tensor_tensor(
                          av2[:, 8:16], ps[:, 512:520],
                          av2[:, 0:8], mybir.AluOpType.subtract,
                      )
                      nc.vector.memset(
                          hrow[:, C2 + (32 if FP8 else 16) : ROW2], 0.0)
                      nc.sync.dma_start(h2_stage[t * 128 : (t + 1) * 128, :], hrow[:])
                nc.gpsimd.collective_compute(
                    "AllGather", mybir.AluOpType.bypass, replica_groups=rg,
                    ins=[h2_stage.opt()], outs=[H2full.opt()],
                )

              # =============== Stage F: GAT2 aggregation ======================
              _mark(nc, "F:gat2-agg")
              x2gat_T = pp.tile([128, ntile, 128], BF16)
              if stage_cap >= 7:
                  gat_agg(ROW2, C2, H2, 512, 640, 768, x2gat_T,
                          H2full[:], h2_stage)

              # =============== Stage G: GCN2 gemm + AG ========================
              _mark(nc, "G:gcn2-gemm+AG")
              if stage_cap >= 8:
                with (
                  tc.tile_pool(name="gps", bufs=2, space="PSUM") as gps,
                  tc.tile_pool(name="gst", bufs=3) as gst,
              ):
                  for t in range(ntile):
                      ps = gps.tile([128, 64], F32, tag="gps")
                      nc.tensor.matmul(ps[:], x2gat_T[:, t, :], wgcn2_sb[:],
                                       start=True, stop=True)
                      yr = gst.tile([128, 128], BF16, tag="yr2")
                      nc.scalar.copy(yr[:, 0:64], ps[:])
                      nc.vector.memset(yr[:, 64:128], 0.0)
                      nc.sync.dma_start(y2_stage[t * 128 : (t + 1) * 128, :], yr[:])
                nc.gpsimd.collective_compute(
                    "AllGather", mybir.AluOpType.bypass, replica_groups=rg,
                    ins=[y2_stage.opt()], outs=[Y2full.opt()],
                )

              # =============== Stage H: GCN2 aggregation ======================
              _mark(nc, "H:gcn2-agg")
              x2gcn_rows = pp.tile([128, ntile, 64], BF16)
              if stage_cap >= 9:
                  gcn_agg(Y2full[:], 64, 896, x2gcn_T, x2gcn_rows)

              # =============== Stage I: fuse + LN3 + classifier ===============
              _mark(nc, "I:fuse+clf")
              if stage_cap >= 10:
                with (
                  tc.tile_pool(name="ips", bufs=2, space="PSUM") as ips,
                  tc.tile_pool(name="ipt", bufs=2, space="PSUM") as ipt,
                  tc.tile_pool(name="icl", bufs=1, space="PSUM") as icl,
                  tc.tile_pool(name="ism", bufs=3) as ism,
                  tc.tile_pool(name="ift", bufs=1) as ift,
              ):
                  fT_all = ift.tile([64, ntile, 128], BF16)
                  for t in range(ntile):
                      pf = ips.tile([128, 64], F32, tag="ip")
                      nc.tensor.matmul(pf[:], x1gcn_T[:, t, :], wf_sb[:, 0, :],
                                       start=True, stop=False)
                      nc.tensor.matmul(pf[:], x2gcn_T[0:64, t, :],
                                       wf_sb[0:64, 1, :], start=False, stop=True)
                      tf = ism.tile([128, 64], F32, tag="tf")
                      nc.vector.tensor_tensor(
                          tf[:], pf[:], cb[:, 1024:1088], mybir.AluOpType.add
                      )
                      nc.vector.tensor_scalar_max(tf[:], tf[:], 0.0)
                      nc.vector.tensor_tensor(
                          tf[:], tf[:], skip_sb[:, t, :], mybir.AluOpType.add
                      )
                      # LN3 over 64
                      nsum = ism.tile([128, 1], F32, tag="insum")
                      nc.vector.tensor_reduce(
                          nsum[:], tf[:], mybir.AxisListType.X,
                          mybir.AluOpType.add, negate=True,
                      )
                      nmu = ism.tile([128, 1], F32, tag="inmu")
                      nc.scalar.mul(nmu[:], nsum[:], 1.0 / 64)
                      xc = ism.tile([128, 64], F32, tag="ixc")
                      nc.vector.tensor_scalar_add(xc[:], tf[:], nmu[:])
                      sq = ism.tile([128, 64], F32, tag="isq")
                      ss = ism.tile([128, 1], F32, tag="iss")
                      nc.vector.tensor_tensor(
                          sq[:], xc[:], xc[:], mybir.AluOpType.mult
                      )
                      nc.vector.tensor_reduce(
                          ss[:], sq[:], mybir.AxisListType.X,
                          mybir.AluOpType.add,
                      )
                      sd = ism.tile([128, 1], F32, tag="isd")
                      nc.scalar.activation(
                          sd[:], ss[:], mybir.ActivationFunctionType.Sqrt,
                          bias=epsb[:], scale=1.0 / 64,
                      )
                      rstd = ism.tile([128, 1], F32, tag="irstd")
                      nc.vector.reciprocal(rstd[:], sd[:])
                      t3 = ism.tile([128, 64], F32, tag="it3")
                      nc.vector.scalar_tensor_tensor(
                          t3[:], xc[:], rstd[:], cb[:, 1088:1152],
                          mybir.AluOpType.mult, mybir.AluOpType.mult,
                      )
                      fin = ism.tile([128, 64], BF16, tag="fin")
                      nc.vector.tensor_tensor(
                          fin[:], t3[:], cb[:, 1152:1216], mybir.AluOpType.add
                      )
                      # stash transposed fin for the batched classifier
                      ptr = ipt.tile([128, 128], BF16, tag="ptr")
                      nc.tensor.transpose(ptr[0:64, 0:128], fin[:, 0:64], idt[:])
                      nc.vector.tensor_copy(fT_all[:, t, :], ptr[0:64, 0:128])
                  # batched classifier in transposed space, 512-col slices:
                  # relu(Wc1^T fT + b) -> relu(Wc2^T . + b) -> Wc3^T . + b
                  NCOLS = ntile * 128
                  fT_f = fT_all.rearrange("p t n -> p (t n)")
                  orow = ism.tile([8, NCOLS], F32, tag="orow")
                  for c0 in range(0, NCOLS, 512):
                      cs = slice(c0, min(NCOLS, c0 + 512))
                      cw = cs.stop - c0
                      p1 = icl.tile([32, 512], F32, tag="p1")
                      nc.tensor.matmul(p1[:, 0:cw], wc1_sb[:], fT_f[:, cs],
                                       start=True, stop=True)
                      h1b = ism.tile([32, 512], BF16, tag="bh1")
                      nc.vector.tensor_scalar(
                          h1b[:, 0:cw], p1[:, 0:cw], cbT_sb[0:32, 0:1], 0.0,
                          mybir.AluOpType.add, mybir.AluOpType.max,
                      )
                      p2 = icl.tile([16, 512], F32, tag="p2")
                      nc.tensor.matmul(p2[:, 0:cw], wc2_sb[:], h1b[:, 0:cw],
                                       start=True, stop=True)
                      h2b = ism.tile([16, 512], BF16, tag="bh2")
                      nc.vector.tensor_scalar(
                          h2b[:, 0:cw], p2[:, 0:cw], cbT_sb[0:16, 1:2], 0.0,
                          mybir.AluOpType.add, mybir.AluOpType.max,
                      )
                      p3 = icl.tile([8, 512], F32, tag="p3")
                      nc.tensor.matmul(p3[:, 0:cw], wc3_sb[:], h2b[:, 0:cw],
                                       start=True, stop=True)
                      nc.vector.tensor_scalar_add(
                          orow[:, cs], p3[:, 0:cw], cbT_sb[0:8, 2:3]
                      )
                  nc.sync.dma_start(out[:, :], orow[:])

            pend = emit_A(0)
            for _r in range(repeat):
                cur = pend
                pend = emit_A(_r + 1) if _r + 1 < repeat else None
                emit_rest(_r, *cur)

            _mark(nc, "Z:end")
            if stage_cap < 10:
                with tc.tile_pool(name="fb", bufs=1) as fb:
                    z = fb.tile([8, n_loc_pad], F32)
                    nc.vector.memset(z[:], 0.0)
                    nc.scalar.dma_start(out[:, :], z[:])

    nc.compile()
    return nc


# ----------------------------------------------------------------------------
# Top-level kernel
# ----------------------------------------------------------------------------

_CACHE = {}
STAGE_MARKS = []


def _mark(nc, label):
    try:
        STAGE_MARKS.append((label, int(nc.next_id())))
    except Exception:
        pass


def prepare(inputs, n_nodes=None, stage_cap=99, agg_cap=99, repeat=1,
            fp8=True, gsz=6, mixed=False, gbufs=2, s01sync=False, pobufs=2):
    """Host prep + (cached) program build. Returns (nc, in_maps, n_loc)."""
    x = np.asarray(inputs["x"], np.float32)
    n = x.shape[0] if n_nodes is None else n_nodes
    f_in = x.shape[1]
    assert n % NCORES == 0
    n_loc = n // NCORES
    n_loc_pad = _rup(n_loc, 128)
    k_pad = _rup(f_in, 128)

    g = prep_graph(inputs["edge_index"], n, n_loc, n_loc_pad)
    w = prep_weights(inputs, k_pad)

    cfg_key = (n_loc_pad, k_pad, w["w1c"], tuple(g["K"]), stage_cap, agg_cap,
               repeat, fp8, gsz, mixed, gbufs, s01sync, pobufs)
    if cfg_key not in _CACHE:
        cfg = {
            "n_loc_pad": n_loc_pad,
            "k_pad": k_pad,
            "w1c": w["w1c"],
            "K": g["K"],
            "totch": g["totch"],
            "nidx": g["nidx"],
            "chunk_off": g["chunk_off"],
            "stage_cap": stage_cap,
            "agg_cap": agg_cap,
            "repeat": repeat,
            "fp8": fp8,
            "gsz": gsz,
            "mixed": mixed,
            "gbufs": gbufs,
            "s01sync": s01sync,
            "pobufs": pobufs,
        }
        _CACHE[cfg_key] = build_nc(cfg)
    nc = _CACHE[cfg_key]

    xp = np.zeros((NCORES * n_loc_pad, k_pad), np.float32)
    for c in range(NCORES):
        xp[c * n_loc_pad : c * n_loc_pad + n_loc, :f_in] = x[
            c * n_loc : (c + 1) * n_loc
        ]
    xpb = xp.astype(NPBF16)

    in_maps = []
    for c in range(NCORES):
        xT_loc = np.ascontiguousarray(
            xpb[c * n_loc_pad : (c + 1) * n_loc_pad].T
        )
        in_maps.append(
            {
                "xT": xT_loc,
                "W1": w["W1"],
                "W2": w["W2"],
                "Wgcn1": w["Wgcn1"],
                "Wgcn2": w["Wgcn2"],
                "Wfuse": w["Wfuse"],
                "Wc1": w["Wc1"],
                "Wc2": w["Wc2"],
                "Wc3": w["Wc3"],
                "cblob": w["cblob"],
                "cbT": w["cbT"],
                "ident": w["ident"],
                "idx16": g["idx16"][c],
                "s01": g["s01"][c],
                "s01T": g["s01T"][c],
                "sg": g["sg"][c],
            }
        )
    return nc, in_maps, n_loc, n_loc_pad


def kernel(**inputs):
    nc, in_maps, n_loc, n_loc_pad = prepare(inputs)
    res = run_bass_kernel_spmd(nc, in_maps, core_ids=list(range(NCORES)))
    n = np.asarray(inputs["x"]).shape[0]
    out = np.zeros((n, 5), np.float32)
    for c in range(NCORES):
        out[c * n_loc : (c + 1) * n_loc] = res.results[c]["out"][:5, :n_loc].T
    return out


if __name__ == "__main__":
    # quick smoke: tiny random problem shaped like the real one
    rng = np.random.default_rng(0)
    N, E, F_IN = 256, 2048, 96
    ip = {
        "x": rng.standard_normal((N, F_IN), dtype=np.float32),
        "edge_index": rng.integers(0, N, (2, E)),
        "W_gat1": rng.standard_normal((F_IN, 768), dtype=np.float32) * 0.05,
        "a_src1": rng.standard_normal((6, 128), dtype=np.float32) * 0.05,
        "a_dst1": rng.standard_normal((6, 128), dtype=np.float32) * 0.05,
        "b_gat1": np.zeros(128, np.float32),
        "W_gcn1": rng.standard_normal((128, 128), dtype=np.float32) * 0.05,
        "b_gcn1": np.zeros(128, np.float32),
        "W_gat2": rng.standard_normal((128, 512), dtype=np.float32) * 0.05,
        "a_src2": rng.standard_normal((4, 128), dtype=np.float32) * 0.05,
        "a_dst2": rng.standard_normal((4, 128), dtype=np.float32) * 0.05,
        "b_gat2": np.zeros(128, np.float32),
        "W_gcn2": rng.standard_normal((128, 64), dtype=np.float32) * 0.05,
        "b_gcn2": np.zeros(64, np.float32),
        "W_skip": rng.standard_normal((F_IN, 64), dtype=np.float32) * 0.05,
        "b_skip": np.zeros(64, np.float32),
        "W_fuse": rng.standard_normal((192, 64), dtype=np.float32) * 0.05,
        "b_fuse": np.zeros(64, np.float32),
        "W_c1": rng.standard_normal((64, 32), dtype=np.float32) * 0.05,
        "b_c1": np.zeros(32, np.float32),
        "W_c2": rng.standard_normal((32, 16), dtype=np.float32) * 0.05,
        "b_c2": np.zeros(16, np.float32),
        "W_c3": rng.standard_normal((16, 5), dtype=np.float32) * 0.05,
        "b_c3": np.zeros(5, np.float32),
        "g1": np.ones(128, np.float32), "be1": np.zeros(128, np.float32),
        "g2": np.ones(128, np.float32), "be2": np.zeros(128, np.float32),
        "g3": np.ones(64, np.float32), "be3": np.zeros(64, np.float32),
    }
    t0 = time.time()
    outv = kernel(**ip)
    print("kernel ran in", time.time() - t0, "shape", outv.shape)
    print(outv[:4])



# revision 20
# speedup vs baseline: 3.0360x; 1.3371x over previous
"""Trainium2 Bass kernel for nn_AdvancedCardiomyocyteGNN (GAT/GCN message passing).

Strategy (8 NeuronCores, SPMD single NEFF):
  - Nodes sharded across cores (1250 -> padded 1280 per core).
  - Node-wise GEMMs computed on the owning core; per-edge alpha projections
    (h . a_src / h . a_dst) are folded into the main GEMM weights on the host.
  - AllGather replicates the transformed node features (bf16) to all cores.
  - Edges partitioned by dst, sorted, grouped per 128-dst block; source rows
    are fetched with dma_gather (128 edges per chunk land on 128 partitions);
    segment softmax + weighted segment sum are computed as one-hot matmuls
    (S^T @ M accumulated in PSUM per dst block).
  - Graph-structure-dependent one-hot/scatter matrices and index tables are
    precomputed on the host (pure preprocessing of the integer edge list).
"""

import sys
import time

sys.path.insert(0, "/opt/trn_rl_repo")

import numpy as np
import ml_dtypes

import concourse.bass as bass
import concourse.tile as tile
from concourse import bacc, mybir
from concourse.bass_utils import run_bass_kernel_spmd

F32 = mybir.dt.float32
BF16 = mybir.dt.bfloat16
F8 = mybir.dt.float8e4
I16 = mybir.dt.int16
NPBF16 = ml_dtypes.bfloat16

NCORES = 8


def _rup(x, m):
    return (x + m - 1) // m * m


# ----------------------------------------------------------------------------
# Host-side graph preprocessing
# ----------------------------------------------------------------------------

def prep_graph(edge_index, n_nodes, n_loc, n_loc_pad, heads_dummy=None):
    """Partition edges (with self loops) by dst across cores, sort by dst,
    group per 128-dst block, pad each block to a per-block common chunk count.

    Returns dict with per-core index/scatter data and layout constants."""
    src = np.asarray(edge_index[0], dtype=np.int64)
    dst = np.asarray(edge_index[1], dtype=np.int64)
    loop = np.arange(n_nodes, dtype=np.int64)
    src = np.concatenate([src, loop])
    dst = np.concatenate([dst, loop])

    # gcn normalization (reference: deg over dst including self loops)
    deg = np.bincount(dst, minlength=n_nodes).astype(np.float64)
    dinv = np.where(deg > 0, deg ** -0.5, 0.0)
    ce_all = (dinv[src] * dinv[dst]).astype(np.float32)

    # padded node ids
    def pad_id(n):
        return (n // n_loc) * n_loc_pad + (n % n_loc)

    srcp = pad_id(src)
    dstp = pad_id(dst)

    core_of = dst // n_loc
    nblk = n_loc_pad // 128

    # per (core, blk) edge lists
    per_core = []
    for c in range(NCORES):
        m = core_of == c
        s, d, ce = srcp[m], dstp[m], ce_all[m]
        dloc = d - c * n_loc_pad
        order = np.argsort(dloc, kind="stable")
        s, dloc, ce = s[order], dloc[order], ce[order]
        blk = dloc // 128
        per_core.append((s, dloc, ce, blk))

    # per-block chunk count, common across cores
    K = np.zeros(nblk, dtype=np.int64)
    for c in range(NCORES):
        _, _, _, blk = per_core[c]
        cnt = np.bincount(blk, minlength=nblk)
        K = np.maximum(K, (cnt + 127) // 128)
    K = np.maximum(K, 1).astype(int)
    totch = int(K.sum())
    nidx = totch * 128

    idx16 = np.zeros((NCORES, 128, nidx // 16), dtype=np.int16)
    s01 = np.zeros((NCORES, 128, totch * 128), dtype=NPBF16)
    s01T = np.zeros((NCORES, 128, totch * 128), dtype=NPBF16)
    sg = np.zeros((NCORES, 128, totch * 128), dtype=NPBF16)

    chunk_off = np.concatenate([[0], np.cumsum(K)])  # chunk offset per block

    for c in range(NCORES):
        s, dloc, ce, blk = per_core[c]
        idx_flat = np.zeros(nidx, dtype=np.int16)
        for b in range(nblk):
            m = blk == b
            sb_, db_, cb_ = s[m], dloc[m] - b * 128, ce[m]
            ne = len(sb_)
            base = chunk_off[b] * 128  # edge slot offset
            idx_flat[base : base + ne] = sb_.astype(np.int16)
            ch = base // 128 + np.arange(ne) // 128  # global chunk id
            e_in = np.arange(ne) % 128
            s01[c, e_in, ch * 128 + db_] = NPBF16(1.0)
            s01T[c, db_, ch * 128 + e_in] = NPBF16(1.0)
            sg[c, e_in, ch * 128 + db_] = cb_.astype(NPBF16)
        idx16[c] = np.tile(idx_flat.reshape(-1, 16).T, (8, 1))

    return {
        "K": K,
        "totch": totch,
        "nidx": nidx,
        "chunk_off": chunk_off,
        "idx16": idx16,
        "s01": s01,
        "s01T": s01T,
        "sg": sg,
    }


def prep_weights(ip, k_pad):
    """Fold/concat/cast weights on the host. Returns dict of shared arrays."""
    f32 = np.float32
    w_gat1 = np.asarray(ip["W_gat1"], f32)  # [F_IN, 768]
    a_src1 = np.asarray(ip["a_src1"], f32)  # [6, 128]
    a_dst1 = np.asarray(ip["a_dst1"], f32)
    w_skip = np.asarray(ip["W_skip"], f32)  # [F_IN, 64]
    f_in = w_gat1.shape[0]
    h1 = a_src1.shape[0]
    c1 = a_src1.shape[1]
    ws1 = np.einsum("khc,hc->kh", w_gat1.reshape(f_in, h1, c1), a_src1)
    wd1 = np.einsum("khc,hc->kh", w_gat1.reshape(f_in, h1, c1), a_dst1)
    w1 = np.concatenate([w_gat1, w_skip, ws1, wd1], axis=1)  # [F_IN, 844]
    w1c = _rup(w1.shape[1], 16)
    w1p = np.zeros((k_pad, w1c), NPBF16)
    w1p[:f_in, : w1.shape[1]] = w1.astype(NPBF16)

    w_gat2 = np.asarray(ip["W_gat2"], f32)  # [128, 512]
    a_src2 = np.asarray(ip["a_src2"], f32)  # [4, 128]
    a_dst2 = np.asarray(ip["a_dst2"], f32)
    h2 = a_src2.shape[0]
    ws2 = np.einsum("khc,hc->kh", w_gat2.reshape(128, h2, c1), a_src2)
    wd2 = np.einsum("khc,hc->kh", w_gat2.reshape(128, h2, c1), a_dst2)
    w2 = np.concatenate([w_gat2, ws2, wd2], axis=1)  # [128, 520]
    w2p = w2.astype(NPBF16)

    def rep(v, cols=None):
        v = np.asarray(v, f32).reshape(-1)
        if cols is not None:
            vv = np.zeros(cols, f32)
            vv[: len(v)] = v
            v = vv
        return np.tile(v[None, :], (128, 1)).astype(f32)

    cblob = np.concatenate(
        [
            rep(ip["b_gat1"]),   # 0:128
            rep(ip["g1"]),       # 128:256
            rep(ip["be1"]),      # 256:384
            rep(ip["b_gcn1"]),   # 384:512
            rep(ip["b_gat2"]),   # 512:640
            rep(ip["g2"]),       # 640:768
            rep(ip["be2"]),      # 768:896
            rep(ip["b_gcn2"], 64),   # 896:960
            rep(ip["b_skip"], 64),   # 960:1024
            rep(ip["b_fuse"], 64),   # 1024:1088
            rep(ip["g3"], 64),       # 1088:1152
            rep(ip["be3"], 64),      # 1152:1216
            rep(ip["b_c1"], 32),     # 1216:1248
            rep(ip["b_c2"], 16),     # 1248:1264
            rep(ip["b_c3"], 8),      # 1264:1272
        ],
        axis=1,
    )

    wf = np.asarray(ip["W_fuse"], f32)  # [192, 64]
    wc1 = np.asarray(ip["W_c1"], f32)  # [64, 32]
    wc2 = np.asarray(ip["W_c2"], f32)  # [32, 16]
    wc3 = np.asarray(ip["W_c3"], f32)  # [16, 5]
    wc3p = np.zeros((wc3.shape[0], 8), np.float32)
    wc3p[:, : wc3.shape[1]] = wc3

    cbT = np.zeros((128, 4), f32)
    cbT[:32, 0] = np.asarray(ip["b_c1"], f32)
    cbT[:16, 1] = np.asarray(ip["b_c2"], f32)
    cbT[:5, 2] = np.asarray(ip["b_c3"], f32)

    return {
        "W1": w1p,
        "W2": w2p,
        "Wgcn1": np.asarray(ip["W_gcn1"], f32).astype(NPBF16),
        "Wgcn2": np.asarray(ip["W_gcn2"], f32).astype(NPBF16),
        "Wfuse": wf.astype(NPBF16),
        "Wc1": wc1.astype(NPBF16),
        "Wc2": wc2.astype(NPBF16),
        "Wc3": wc3p.astype(NPBF16),
        "cblob": cblob,
        "cbT": cbT,
        "ident": np.eye(128, dtype=NPBF16),
        "w1c": w1c,
    }


# ----------------------------------------------------------------------------
# Bass program builder
# ----------------------------------------------------------------------------

def build_nc(cfg):
    """cfg: dict with n_loc_pad, k_pad (F_IN padded), K (list per block),
    totch, nidx, w1c, h1=6, h2=4."""
    n_loc_pad = cfg["n_loc_pad"]
    k_pad = cfg["k_pad"]
    Kb = cfg["K"]
    totch = cfg["totch"]
    nidx = cfg["nidx"]
    w1c = cfg["w1c"]
    chunk_off = cfg["chunk_off"]
    nblk = n_loc_pad // 128
    ntile = nblk
    kch = k_pad // 128
    NP = NCORES * n_loc_pad
    H1, H2 = 6, 4
    FP8 = bool(cfg.get("fp8", False))
    GSZ = int(cfg.get("gsz", 6))
    MIXED = bool(cfg.get("mixed", False))
    GB = int(cfg.get("gbufs", 2))
    S01SYNC = bool(cfg.get("s01sync", False))
    POB = int(cfg.get("pobufs", 2))
    HD = F8 if FP8 else BF16
    if FP8:
        # fp8 rows: [h fp8 | alpha hi/lo as raw bf16 bytes | pad to 256B]
        ROW1 = _rup(H1 * 128 + 2 * 24, 256)   # 1024
        ROW2 = _rup(H2 * 128 + 2 * 16, 256)   # 768
    else:
        ROW1 = _rup(H1 * 128 + 24, 128)   # h(768)+asrc hi/lo+adst hi/lo
        ROW2 = _rup(H2 * 128 + 16, 128)
    C1 = H1 * 128
    C2 = H2 * 128
    AS1 = C1 + 24   # alpha block end (asrc hi/lo + adst hi/lo)
    AS2 = C2 + 16
    EPS = 1e-5

    stage_cap = cfg.get("stage_cap", 99)
    agg_cap = cfg.get("agg_cap", 99)
    repeat = cfg.get("repeat", 1)
    STAGE_MARKS.clear()
    nc = bacc.Bacc("TRN2", target_bir_lowering=False, debug=False,
                   num_devices=NCORES, num_swdge_queues=4)

    def din(name, shape, dt):
        return nc.dram_tensor(name, shape, dt, kind="ExternalInput").ap()

    xT = din("xT", [k_pad, n_loc_pad], BF16)
    W1 = din("W1", [k_pad, w1c], BF16)
    W2 = din("W2", [128, 520], BF16)
    Wgcn1 = din("Wgcn1", [128, 128], BF16)
    Wgcn2 = din("Wgcn2", [128, 64], BF16)
    Wfuse = din("Wfuse", [192, 64], BF16)
    Wc1 = din("Wc1", [64, 32], BF16)
    Wc2 = din("Wc2", [32, 16], BF16)
    Wc3 = din("Wc3", [16, 8], BF16)
    cblob = din("cblob", [128, 1272], F32)
    ident = din("ident", [128, 128], BF16)
    idx16 = din("idx16", [128, nidx // 16], I16)
    s01b = din("s01", [128, totch * 128], BF16)
    s01Tb = din("s01T", [128, totch * 128], BF16)
    sgb = din("sg", [128, totch * 128], BF16)

    cbT = din("cbT", [128, 4], F32)
    out = nc.dram_tensor("out", [8, n_loc_pad], F32, kind="ExternalOutput").ap()

    rg = [list(range(NCORES))]

    _qctr = [0]

    def _qn():
        _qctr[0] = (_qctr[0] + 1) % 4
        return _qctr[0]

    with tile.TileContext(nc) as tc:
        with (
            tc.tile_pool(name="const", bufs=1) as cpool,
            tc.tile_pool(name="persist", bufs=2) as pp,
            tc.tile_pool(name="xtp", bufs=3) as axp,
            tc.tile_pool(name="dram", bufs=1, space="DRAM") as dpool,
        ):
            # ---- constants / persistent tiles ----
            cb = cpool.tile([128, 1272], F32)
            nc.scalar.dma_start(cb[:], cblob[:])
            idt = cpool.tile([128, 128], BF16)
            nc.scalar.dma_start(idt[:], ident[:])
            idxs = cpool.tile([128, nidx // 16], I16)
            nc.scalar.dma_start(idxs[:], idx16[:])
            wgcn1_sb = cpool.tile([128, 128], BF16)
            nc.scalar.dma_start(wgcn1_sb[:], Wgcn1[:])
            wgcn2_sb = cpool.tile([128, 64], BF16)
            nc.scalar.dma_start(wgcn2_sb[:], Wgcn2[:])
            w2_sb = cpool.tile([128, 520], BF16)
            nc.scalar.dma_start(w2_sb[:], W2[:])
            wf_sb = cpool.tile([128, 2, 64], BF16)
            nc.scalar.dma_start(wf_sb[:, 0, :], Wfuse[0:128, :])
            nc.scalar.dma_start(wf_sb[:64, 1, :], Wfuse[128:192, :])
            wc1_sb = cpool.tile([64, 32], BF16)
            nc.scalar.dma_start(wc1_sb[:], Wc1[:])
            wc2_sb = cpool.tile([32, 16], BF16)
            nc.scalar.dma_start(wc2_sb[:], Wc2[:])
            wc3_sb = cpool.tile([16, 8], BF16)
            nc.scalar.dma_start(wc3_sb[:], Wc3[:])
            epsb = cpool.tile([128, 1], F32)
            nc.vector.memset(epsb[:], EPS)
            cbT_sb = cpool.tile([128, 4], F32)
            nc.scalar.dma_start(cbT_sb[:], cbT[:])
            # weights for GEMM1 live in SBUF across reps
            w1_sb = cpool.tile([128, kch, w1c], BF16)
            W1_r = W1.rearrange("(c p) n -> p c n", p=128)
            for c in range(kch):
                nc.scalar.dma_start(w1_sb[:, c, :], W1_r[:, c, :])

            # =============== Stage A: GEMM1 (x @ [Wgat1|Wskip|Ws|Wd]) =======
            # Software-pipelined emission: stage A (+ its AllGather) of rep
            # r+1 is emitted BEFORE stages B..I of rep r so the per-engine
            # in-order queues can overlap the next rep's GEMM and AG wire
            # time with the current rep's aggregation work.
            def emit_A(_rep):
              _mark(nc, f"A:gemm1 r{_rep}")
              h1_stage = dpool.tile([n_loc_pad, ROW1], HD, tag=f"h1s{_rep}")
              H1full = dpool.tile([NP, ROW1], HD, addr_space="Shared",
                                  tag=f"H1f{_rep}")
              skip_sb = pp.tile([128, ntile, 64], F32, tag="skip")
              with (
                  tc.tile_pool(name="apsum", bufs=1, space="PSUM") as apsum,
                  tc.tile_pool(name="astage", bufs=3) as astage,
              ):
                  xT_r = xT.rearrange("(c p) n -> p c n", p=128)
                  for t in range(ntile):
                      ns = t * 128
                      xt_sb = axp.tile([128, kch, 128], BF16, tag="xt")
                      nc.sync.dma_start(xt_sb[:], xT_r[:, :, ns : ns + 128])
                      ps = apsum.tile([128, w1c], F32, tag="ps")
                      for c in range(kch):
                          lhsT = xt_sb[:, c, :]
                          nc.tensor.matmul(
                              ps[:, 0:512], lhsT, w1_sb[:, c, 0:512],
                              start=(c == 0), stop=(c == kch - 1),
                          )
                          nc.tensor.matmul(
                              ps[:, 512:w1c], lhsT, w1_sb[:, c, 512:w1c],
                              start=(c == 0), stop=(c == kch - 1),
                          )
                      hrow = astage.tile([128, ROW1], HD, tag="hrow")
                      nc.scalar.copy(hrow[:, 0:C1], ps[:, 0:C1])
                      av = (hrow[:, C1 : C1 + 48].bitcast(BF16) if FP8
                            else hrow[:, C1 : C1 + 24])
                      nc.vector.tensor_copy(
                          av[:, 0:12], ps[:, C1 + 64 : C1 + 76]
                      )
                      nc.vector.tensor_tensor(
                          av[:, 12:24],
                          ps[:, C1 + 64 : C1 + 76],
                          av[:, 0:12], mybir.AluOpType.subtract,
                      )
                      nc.vector.memset(
                          hrow[:, C1 + (48 if FP8 else 24) : ROW1], 0.0)
                      nc.sync.dma_start(h1_stage[ns : ns + 128, :], hrow[:])
                      # skip = relu(x@Wskip + b_skip)
                      tsk = astage.tile([128, 64], F32, tag="tsk")
                      nc.vector.tensor_tensor(
                          tsk[:], ps[:, C1 : C1 + 64], cb[:, 960:1024],
                          mybir.AluOpType.add,
                      )
                      nc.vector.tensor_scalar_max(skip_sb[:, t, :], tsk[:], 0.0)

              _mark(nc, "A2:AG-H1")
              if stage_cap >= 2:
                  nc.gpsimd.collective_compute(
                      "AllGather", mybir.AluOpType.bypass, replica_groups=rg,
                      ins=[h1_stage.opt()], outs=[H1full.opt()],
                  )
              return h1_stage, H1full, skip_sb

            if True:
              # =============== helper: GAT aggregation stage ==================
              def gat_agg(row, ch, nh, b_off, g_off, be_off,
                          out_T, src_full_ap, stage_ap):
                  """Per dst-block: gather rows, softmax-weighted segment sum,
                  head mean + bias + LN + relu; writes [nodes,128] bf16 blocks
                  transposed into out_T."""
                  kmaxb = int(max(Kb))
                  with (
                      tc.tile_pool(name=f"g{nh}", bufs=GB) as gp,
                      tc.tile_pool(name=f"gd{nh}", bufs=3) as gdp,
                      tc.tile_pool(name=f"m{nh}", bufs=2) as mp,
                      tc.tile_pool(name=f"s{nh}", bufs=2) as sp,
                      tc.tile_pool(name=f"sml{nh}", bufs=2) as sml,
                      tc.tile_pool(name=f"po{nh}", bufs=POB, space="PSUM") as pop,
                      tc.tile_pool(name=f"pt{nh}", bufs=2, space="PSUM") as ptp,
                      tc.tile_pool(name=f"pa{nh}", bufs=2, space="PSUM") as pap,
                  ):
                      for b in range(nblk):
                          K = int(Kb[b])
                          co = int(chunk_off[b])
                          G = gp.tile([128, kmaxb, row], HD, tag="G")
                          for c0 in range(0, K, 8):
                              kk = min(8, K - c0)
                              nc.gpsimd.dma_gather(
                                  G[:, c0 : c0 + kk, :], src_full_ap,
                                  idxs[:, (co + c0) * 8 : (co + c0 + kk) * 8],
                                  num_idxs=kk * 128, num_idxs_reg=kk * 128,
                                  elem_size=row, elem_step=row,
                                  queue_num=_qn(),
                              )
                          # dst-alpha block: the dst rows of block b are the
                          # core's OWN stage rows (local, pre-collective);
                          # broadcast dst alpha to edge slots via per-chunk
                          # matmul with the transposed one-hot s01T.
                          acols = 8 * nh if FP8 else 4 * nh
                          adst = gdp.tile([128, acols], HD, tag="adst")
                          nc.sync.dma_start(
                              adst[:],
                              stage_ap[b * 128 : (b + 1) * 128, ch : ch + acols],
                          )
                          adst_bf = (adst.bitcast(BF16) if FP8 else adst)
                          s01T_sb = gdp.tile([128, kmaxb, 128], BF16, tag="s01T")
                          nc.sync.dma_start(
                              s01T_sb[:, 0:K, :],
                              s01Tb[:, co * 128 : (co + K) * 128],
                          )
                          ald = pap.tile([128, kmaxb, 4 * nh], F32, tag="ald")
                          for c in range(K):
                              nc.tensor.matmul(
                                  ald[:, c, :], s01T_sb[:, c, :],
                                  adst_bf[:, 0 : 4 * nh],
                                  start=True, stop=True,
                              )
                          # bf16 views of the alpha cols (raw bytes when fp8)
                          Gav = (G[:, :, ch : ch + 8 * nh].bitcast(BF16)
                                 if FP8 else G[:, :, ch : ch + 4 * nh])
                          Gdav = ald
                          if agg_cap < 2:
                              continue
                          s01_sb = sp.tile([128, kmaxb, 128], BF16, tag="s01")
                          nc.sync.dma_start(
                              s01_sb[:, 0:K, :], s01b[:, co * 128 : (co + K) * 128]
                          )
                          if agg_cap < 4:
                              continue
                          # alpha = lrelu(asrc_src + adst_dst); e = exp(alpha)
                          # processed in sub-groups of GSZ chunks so PE can
                          # start aggregating while DVE scales later groups
                          al = sml.tile([128, kmaxb, nh], F32, tag="al")
                          e_sb = sml.tile([128, kmaxb, nh], F32, tag="e")
                          e_dup = sml.tile([128, kmaxb, nh, 2], BF16,
                                           tag="edup")
                          po = pop.tile([128, ch + 8], F32, tag="po")
                          # sub-group size GSZ from cfg; M rotates per group
                          for g0 in range(0, K, GSZ):
                              g1 = min(K, g0 + GSZ)
                              gs = slice(g0, g1)
                              gn_ = g1 - g0
                              gl = slice(0, gn_)
                              M = mp.tile([128, GSZ, ch + 8], BF16, tag="M")
                              nc.vector.tensor_tensor(
                                  al[:, gs, :], Gav[:, gs, 0:nh],
                                  Gav[:, gs, 2 * nh : 3 * nh],
                                  mybir.AluOpType.add,
                              )
                              nc.vector.tensor_tensor(
                                  al[:, gs, :], al[:, gs, :],
                                  Gdav[:, gs, nh : 2 * nh],
                                  mybir.AluOpType.add,
                              )
                              nc.vector.tensor_tensor(
                                  al[:, gs, :], al[:, gs, :],
                                  Gdav[:, gs, 3 * nh : 4 * nh],
                                  mybir.AluOpType.add,
                              )
                              nc.vector.scalar_tensor_tensor(
                                  al[:, gs, :], al[:, gs, :], 0.2,
                                  al[:, gs, :],
                                  mybir.AluOpType.mult, mybir.AluOpType.max,
                              )
                              if agg_cap < 5:
                                  continue
                              nc.scalar.activation(
                                  e_sb[:, gs, :], al[:, gs, :],
                                  mybir.ActivationFunctionType.Exp,
                              )
                              nc.vector.tensor_copy(
                                  M[:, gl, ch : ch + nh], e_sb[:, gs, :]
                              )
                              # scaled messages; pair-duplicated e keeps the
                              # DVE tensor_tensor in 2x_1P mode
                              nc.vector.tensor_copy(
                                  e_dup[:, gs],
                                  e_sb[:, gs].unsqueeze(3).broadcast_to(
                                      [128, gn_, nh, 2]
                                  ),
                              )
                              if FP8 and not MIXED:
                                  # upconvert on ACT, scale in place on DVE
                                  nc.scalar.copy(
                                      M[:, gl, 0:ch], G[:, gs, 0:ch]
                                  )
                                  min_ = M[:, gl, 0:ch]
                              else:
                                  # bf16, or mixed fp8xbf16 DVE read
                                  min_ = G[:, gs, 0:ch]
                              nc.vector.tensor_tensor(
                                  M[:, gl, 0:ch].rearrange(
                                      "p k (h q t) -> p k h q t", h=nh, t=2
                                  ),
                                  min_.rearrange(
                                      "p k (h q t) -> p k h q t", h=nh, t=2
                                  ),
                                  e_dup[:, gs].unsqueeze(3).broadcast_to(
                                      [128, gn_, nh, 64, 2]
                                  ),
                                  mybir.AluOpType.mult,
                              )
                              if agg_cap < 6:
                                  continue
                              # aggregate (last nh cols accumulate the
                              # denominators)
                              for c in range(g0, g1):
                                  first, last = c == 0, c == K - 1
                                  nc.tensor.matmul(
                                      po[:, 0:512], s01_sb[:, c, :],
                                      M[:, c - g0, 0:512],
                                      start=first, stop=last,
                                  )
                                  nc.tensor.matmul(
                                      po[:, 512 : ch + nh], s01_sb[:, c, :],
                                      M[:, c - g0, 512 : ch + nh],
                                      start=first, stop=last,
                                  )
                          if agg_cap < 5 or agg_cap < 6:
                              continue
                          if agg_cap < 61:
                              continue
                          # normalize + head mean + bias + LN + relu
                          den = sml.tile([128, nh], F32, tag="den")
                          nc.vector.tensor_scalar_max(
                              den[:], po[:, ch : ch + nh], 1e-30
                          )
                          rden = sml.tile([128, nh], F32, tag="rden")
                          nc.vector.reciprocal(rden[:], den[:])
                          if agg_cap < 62:
                              continue
                          gn = sml.tile([128, ch], F32, tag="gn")
                          nc.vector.tensor_tensor(
                              gn.rearrange("p (h c) -> p h c", c=128),
                              po[:, 0:ch].rearrange("p (h c) -> p h c", c=128),
                              rden.unsqueeze(2).broadcast_to([128, nh, 128]),
                              mybir.AluOpType.mult,
                          )
                          if agg_cap < 63:
                              continue
                          hm = sml.tile([128, 128], F32, tag="hm")
                          if nh == 6:
                              t2 = sml.tile([128, 384], F32, tag="t2")
                              nc.vector.tensor_tensor(
                                  t2[:], gn[:, 0:384], gn[:, 384:768],
                                  mybir.AluOpType.add,
                              )
                              nc.vector.tensor_tensor(
                                  hm[:], t2[:, 0:128], t2[:, 128:256],
                                  mybir.AluOpType.add,
                              )
                              nc.vector.tensor_tensor(
                                  hm[:], hm[:], t2[:, 256:384],
                                  mybir.AluOpType.add,
                              )
                          else:
                              t2 = sml.tile([128, 256], F32, tag="t2")
                              nc.vector.tensor_tensor(
                                  t2[:], gn[:, 0:256], gn[:, 256:512],
                                  mybir.AluOpType.add,
                              )
                              nc.vector.tensor_tensor(
                                  hm[:], t2[:, 0:128], t2[:, 128:256],
                                  mybir.AluOpType.add,
                              )
                          # hm = hm/nh + bias
                          nc.vector.scalar_tensor_tensor(
                              hm[:], hm[:], 1.0 / nh, cb[:, b_off : b_off + 128],
                              mybir.AluOpType.mult, mybir.AluOpType.add,
                          )
                          if agg_cap < 64:
                              continue
                          # LayerNorm over 128
                          nsum = sml.tile([128, 1], F32, tag="nsum")
                          nc.vector.tensor_reduce(
                              nsum[:], hm[:], mybir.AxisListType.X,
                              mybir.AluOpType.add, negate=True,
                          )
                          nmu = sml.tile([128, 1], F32, tag="nmu")
                          nc.scalar.mul(nmu[:], nsum[:], 1.0 / 128)
                          xc = sml.tile([128, 128], F32, tag="xc")
                          nc.vector.tensor_scalar_add(xc[:], hm[:], nmu[:])
                          if agg_cap < 65:
                              continue
                          sq = sml.tile([128, 128], F32, tag="sq")
                          ss = sml.tile([128, 1], F32, tag="ss")
                          nc.vector.tensor_tensor(
                              sq[:], xc[:], xc[:], mybir.AluOpType.mult
                          )
                          nc.vector.tensor_reduce(
                              ss[:], sq[:], mybir.AxisListType.X,
                              mybir.AluOpType.add,
                          )
                          if agg_cap < 66:
                              continue
                          sd = sml.tile([128, 1], F32, tag="sd")
                          nc.scalar.activation(
                              sd[:], ss[:], mybir.ActivationFunctionType.Sqrt,
                              bias=epsb[:], scale=1.0 / 128,
                          )
                          rstd = sml.tile([128, 1], F32, tag="rstd")
                          nc.vector.reciprocal(rstd[:], sd[:])
                          if agg_cap < 67:
                              continue
                          t3 = sml.tile([128, 128], F32, tag="t3")
                          nc.vector.scalar_tensor_tensor(
                              t3[:], xc[:], rstd[:], cb[:, g_off : g_off + 128],
                              mybir.AluOpType.mult, mybir.AluOpType.mult,
                          )
                          t4 = sml.tile([128, 128], F32, tag="t4")
                          nc.vector.tensor_tensor(
                              t4[:], t3[:], cb[:, be_off : be_off + 128],
                              mybir.AluOpType.add,
                          )
                          xg = sml.tile([128, 128], BF16, tag="xg")
                          nc.vector.tensor_scalar_max(xg[:], t4[:], 0.0)
                          if agg_cap < 68:
                              continue
                          # transpose for the next GEMM
                          pt = ptp.tile([128, 128], BF16, tag="pt")
                          nc.tensor.transpose(pt[:], xg[:], idt[:])
                          nc.vector.tensor_copy(out_T[:, b, :], pt[:])

              # =============== Stage B: GAT1 aggregation ======================
              _mark(nc, "B:gat1-agg")
              x1gat_T = pp.tile([128, ntile, 128], BF16)
              if stage_cap >= 3:
                  gat_agg(ROW1, C1, H1, 0, 128, 256, x1gat_T,
                          H1full[:], h1_stage)

              # =============== Stage C: GCN1 gemm + AG ========================
              _mark(nc, "C:gcn1-gemm+AG")
              if stage_cap >= 4:
                with (
                  tc.tile_pool(name="cps", bufs=2, space="PSUM") as cps,
                  tc.tile_pool(name="cst", bufs=3) as cst,
              ):
                  for t in range(ntile):
                      ps = cps.tile([128, 128], F32, tag="cps")
                      nc.tensor.matmul(ps[:], x1gat_T[:, t, :], wgcn1_sb[:],
                                       start=True, stop=True)
                      yr = cst.tile([128, 128], BF16, tag="yr")
                      nc.scalar.copy(yr[:], ps[:])
                      nc.sync.dma_start(y1_stage[t * 128 : (t + 1) * 128, :], yr[:])
                nc.gpsimd.collective_compute(
                    "AllGather", mybir.AluOpType.bypass, replica_groups=rg,
                    ins=[y1_stage.opt()], outs=[Y1full.opt()],
                )

              # =============== Stage D: GCN1 aggregation ======================
              def gcn_agg(Yfull_ap, ccols, b_off, out_T, out_rows):
                  kmaxb = int(max(Kb))
                  with (
                      tc.tile_pool(name="gy", bufs=3) as gyp,
                      tc.tile_pool(name="sgp", bufs=3) as sgp,
                      tc.tile_pool(name="dsm", bufs=3) as dsm,
                      tc.tile_pool(name="dpo", bufs=2, space="PSUM") as dpo,
                      tc.tile_pool(name="dpt", bufs=2, space="PSUM") as dpt,
                  ):
                      for b in range(nblk):
                          K = int(Kb[b])
                          co = int(chunk_off[b])
                          Gy = gyp.tile([128, kmaxb, 128], BF16, tag="Gy")
                          for c0 in range(0, K, 8):
                              kk = min(8, K - c0)
                              nc.gpsimd.dma_gather(
                                  Gy[:, c0 : c0 + kk, :], Yfull_ap,
                                  idxs[:, (co + c0) * 8 : (co + c0 + kk) * 8],
                                  num_idxs=kk * 128, num_idxs_reg=kk * 128,
                                  elem_size=128, elem_step=128,
                                  queue_num=_qn(),
                              )
                          sg_sb = sgp.tile([128, kmaxb, 128], BF16, tag="sg")
                          nc.sync.dma_start(
                              sg_sb[:, 0:K, :], sgb[:, co * 128 : (co + K) * 128]
                          )
                          po = dpo.tile([128, ccols], F32, tag="dpo")
                          for c in range(K):
                              nc.tensor.matmul(
                                  po[:], sg_sb[:, c, :], Gy[:, c, 0:ccols],
                                  start=(c == 0), stop=(c == K - 1),
                              )
                          t5 = dsm.tile([128, ccols], F32, tag="t5")
                          nc.vector.tensor_tensor(
                              t5[:], po[:], cb[:, b_off : b_off + ccols],
                              mybir.AluOpType.add,
                          )
                          xg = dsm.tile([128, ccols], BF16, tag="xgc")
                          nc.vector.tensor_scalar_max(xg[:], t5[:], 0.0)
                          if out_rows is not None:
                              nc.vector.tensor_copy(out_rows[:, b, :], xg[:])
                          pt = dpt.tile([128, 128], BF16, tag="dpt")
                          nc.tensor.transpose(
                              pt[0:ccols, 0:128], xg[:, 0:ccols], idt[:]
                          )
                          nc.vector.tensor_copy(
                              out_T[0:ccols, b, :], pt[0:ccols, 0:128]
                          )

              _mark(nc, "D:gcn1-agg")
              if stage_cap >= 5:
                  gcn_agg(Y1full[:], 128, 384, x1gcn_T, None)

              # =============== Stage E: GAT2 gemm + AG ========================
              _mark(nc, "E:gat2-gemm+AG")
              if stage_cap >= 6:
                with (
                  tc.tile_pool(name="eps", bufs=2, space="PSUM") as epsp,
                  tc.tile_pool(name="est", bufs=3) as est,
              ):
                  for t in range(ntile):
                      ps = epsp.tile([128, 520], F32, tag="eps")
                      nc.tensor.matmul(ps[:, 0:512], x1gcn_T[:, t, :],
                                       w2_sb[:, 0:512], start=True, stop=True)
                      nc.tensor.matmul(ps[:, 512:520], x1gcn_T[:, t, :],
                                       w2_sb[:, 512:520], start=True, stop=True)
                      hrow = est.tile([128, ROW2], HD, tag="hrow2")
                      nc.scalar.copy(hrow[:, 0:C2], ps[:, 0:C2])
                      av2 = (hrow[:, C2 : C2 + 32].bitcast(BF16) if FP8
                             else hrow[:, C2 : C2 + 16])
                      nc.vector.tensor_copy(av2[:, 0:8], ps[:, 512:520])
                      nc.vector.tensor_tensor(
                          av2[:, 8:16], ps[:, 512:520],
                          av2[:, 0:8], mybir.AluOpType.subtract,
                      )
                      nc.vector.memset(
                          hrow[:, C2 + (32 if FP8 else 16) : ROW2], 0.0)
                      nc.sync.dma_start(h2_stage[t * 128 : (t + 1) * 128, :], hrow[:])
                nc.gpsimd.collective_compute(
                    "AllGather", mybir.AluOpType.bypass, replica_groups=rg,
                    ins=[h2_stage.opt()], outs=[H2full.opt()],
                )

              # =============== Stage F: GAT2 aggregation ======================
              _mark(nc, "F:gat2-agg")
              x2gat_T = pp.tile([128, ntile, 128], BF16)
              if stage_cap >= 7:
                  gat_agg(ROW2, C2, H2, 512, 640, 768, x2gat_T,
                          H2full[:], h2_stage)

              # =============== Stage G: GCN2 gemm + AG ========================
              _mark(nc, "G:gcn2-gemm+AG")
              if stage_cap >= 8:
                with (
                  tc.tile_pool(name="gps", bufs=2, space="PSUM") as gps,
                  tc.tile_pool(name="gst", bufs=3) as gst,
              ):
                  for t in range(ntile):
                      ps = gps.tile([128, 64], F32, tag="gps")
                      nc.tensor.matmul(ps[:], x2gat_T[:, t, :], wgcn2_sb[:],
                                       start=True, stop=True)
                      yr = gst.tile([128, 128], BF16, tag="yr2")
                      nc.scalar.copy(yr[:, 0:64], ps[:])
                      nc.vector.memset(yr[:, 64:128], 0.0)
                      nc.sync.dma_start(y2_stage[t * 128 : (t + 1) * 128, :], yr[:])
                nc.gpsimd.collective_compute(
                    "AllGather", mybir.AluOpType.bypass, replica_groups=rg,
                    ins=[y2_stage.opt()], outs=[Y2full.opt()],
                )

              # =============== Stage H: GCN2 aggregation ======================
              _mark(nc, "H:gcn2-agg")
              x2gcn_rows = pp.tile([128, ntile, 64], BF16)
              if stage_cap >= 9:
                  gcn_agg(Y2full[:], 64, 896, x2gcn_T, x2gcn_rows)

              # =============== Stage I: fuse + LN3 + classifier ===============
              _mark(nc, "I:fuse+clf")
              if stage_cap >= 10:
                with (
                  tc.tile_pool(name="ips", bufs=2, space="PSUM") as ips,
                  tc.tile_pool(name="ipt", bufs=2, space="PSUM") as ipt,
                  tc.tile_pool(name="icl", bufs=1, space="PSUM") as icl,
                  tc.tile_pool(name="ism", bufs=3) as ism,
                  tc.tile_pool(name="ift", bufs=1) as ift,
              ):
                  fT_all = ift.tile([64, ntile, 128], BF16)
                  for t in range(ntile):
                      pf = ips.tile([128, 64], F32, tag="ip")
                      nc.tensor.matmul(pf[:], x1gcn_T[:, t, :], wf_sb[:, 0, :],
                                       start=True, stop=False)
                      nc.tensor.matmul(pf[:], x2gcn_T[0:64, t, :],
                                       wf_sb[0:64, 1, :], start=False, stop=True)
                      tf = ism.tile([128, 64], F32, tag="tf")
                      nc.vector.tensor_tensor(
                          tf[:], pf[:], cb[:, 1024:1088], mybir.AluOpType.add
                      )
                      nc.vector.tensor_scalar_max(tf[:], tf[:], 0.0)
                      nc.vector.tensor_tensor(
                          tf[:], tf[:], skip_sb[:, t, :], mybir.AluOpType.add
                      )
                      # LN3 over 64
                      nsum = ism.tile([128, 1], F32, tag="insum")
                      nc.vector.tensor_reduce(
                          nsum[:], tf[:], mybir.AxisListType.X,
                          mybir.AluOpType.add, negate=True,
                      )
                      nmu = ism.tile([128, 1], F32, tag="inmu")
                      nc.scalar.mul(nmu[:], nsum[:], 1.0 / 64)
                      xc = ism.tile([128, 64], F32, tag="ixc")
                      nc.vector.tensor_scalar_add(xc[:], tf[:], nmu[:])
                      sq = ism.tile([128, 64], F32, tag="isq")
                      ss = ism.tile([128, 1], F32, tag="iss")
                      nc.vector.tensor_tensor(
                          sq[:], xc[:], xc[:], mybir.AluOpType.mult
                      )
                      nc.vector.tensor_reduce(
                          ss[:], sq[:], mybir.AxisListType.X,
                          mybir.AluOpType.add,
                      )
                      sd = ism.tile([128, 1], F32, tag="isd")
                      nc.scalar.activation(
                          sd[:], ss[:], mybir.ActivationFunctionType.Sqrt,
                          bias=epsb[:], scale=1.0 / 64,
                      )
                      rstd = ism.tile([128, 1], F32, tag="irstd")
                      nc.vector.reciprocal(rstd[:], sd[:])
                      t3 = ism.tile([128, 64], F32, tag="it3")
                      nc.vector.scalar_tensor_tensor(
                          t3[:], xc[:], rstd[:], cb[:, 1088:1152],
                          mybir.AluOpType.mult, mybir.AluOpType.mult,
                      )
                      fin = ism.tile([128, 64], BF16, tag="fin")
                      nc.vector.tensor_tensor(
                          fin[:], t3[:], cb[:, 1152:1216], mybir.AluOpType.add
                      )
                      # stash transposed fin for the batched classifier
                      ptr = ipt.tile([128, 128], BF16, tag="ptr")
                      nc.tensor.transpose(ptr[0:64, 0:128], fin[:, 0:64], idt[:])
                      nc.vector.tensor_copy(fT_all[:, t, :], ptr[0:64, 0:128])
                  # batched classifier in transposed space, 512-col slices:
                  # relu(Wc1^T fT + b) -> relu(Wc2^T . + b) -> Wc3^T . + b
                  NCOLS = ntile * 128
                  fT_f = fT_all.rearrange("p t n -> p (t n)")
                  orow = ism.tile([8, NCOLS], F32, tag="orow")
                  for c0 in range(0, NCOLS, 512):
                      cs = slice(c0, min(NCOLS, c0 + 512))
                      cw = cs.stop - c0
                      p1 = icl.tile([32, 512], F32, tag="p1")
                      nc.tensor.matmul(p1[:, 0:cw], wc1_sb[:], fT_f[:, cs],
                                       start=True, stop=True)
                      h1b = ism.tile([32, 512], BF16, tag="bh1")
                      nc.vector.tensor_scalar(
                          h1b[:, 0:cw], p1[:, 0:cw], cbT_sb[0:32, 0:1], 0.0,
                          mybir.AluOpType.add, mybir.AluOpType.max,
                      )
                      p2 = icl.tile([16, 512], F32, tag="p2")
                      nc.tensor.matmul(p2[:, 0:cw], wc2_sb[:], h1b[:, 0:cw],
                                       start=True, stop=True)
                      h2b = ism.tile([16, 512], BF16, tag="bh2")
                      nc.vector.tensor_scalar(
                          h2b[:, 0:cw], p2[:, 0:cw], cbT_sb[0:16, 1:2], 0.0,
                          mybir.AluOpType.add, mybir.AluOpType.max,
                      )
                      p3 = icl.tile([8, 512], F32, tag="p3")
                      nc.tensor.matmul(p3[:, 0:cw], wc3_sb[:], h2b[:, 0:cw],
                                       start=True, stop=True)
                      nc.vector.tensor_scalar_add(
                          orow[:, cs], p3[:, 0:cw], cbT_sb[0:8, 2:3]
                      )
                  nc.sync.dma_start(out[:, :], orow[:])

            pend = emit_A(0)
            for _r in range(repeat):
                cur = pend
                pend = emit_A(_r + 1) if _r + 1 < repeat else None
                emit_rest(_r, *cur)

            _mark(nc, "Z:end")
            if stage_cap < 10:
                with tc.tile_pool(name="fb", bufs=1) as fb:
                    z = fb.tile([8, n_loc_pad], F32)
                    nc.vector.memset(z[:], 0.0)
                    nc.scalar.dma_start(out[:, :], z[:])

    nc.compile()
    return nc


# ----------------------------------------------------------------------------
# Top-level kernel
# ----------------------------------------------------------------------------

_CACHE = {}
STAGE_MARKS = []


def _mark(nc, label):
    try:
        STAGE_MARKS.append((label, int(nc.next_id())))
    except Exception:
        pass


def prepare(inputs, n_nodes=None, stage_cap=99, agg_cap=99, repeat=1,
            fp8=True, gsz=6, mixed=False, gbufs=2, s01sync=False, pobufs=2):
    """Host prep + (cached) program build. Returns (nc, in_maps, n_loc)."""
    x = np.asarray(inputs["x"], np.float32)
    n = x.shape[0] if n_nodes is None else n_nodes
    f_in = x.shape[1]
    assert n % NCORES == 0
    n_loc = n // NCORES
    n_loc_pad = _rup(n_loc, 128)
    k_pad = _rup(f_in, 128)

    g = prep_graph(inputs["edge_index"], n, n_loc, n_loc_pad)
    w = prep_weights(inputs, k_pad)

    cfg_key = (n_loc_pad, k_pad, w["w1c"], tuple(g["K"]), stage_cap, agg_cap,
               repeat, fp8, gsz, mixed, gbufs, s01sync, pobufs)
    if cfg_key not in _CACHE:
        cfg = {
            "n_loc_pad": n_loc_pad,
            "k_pad": k_pad,
            "w1c": w["w1c"],
            "K": g["K"],
            "totch": g["totch"],
            "nidx": g["nidx"],
            "chunk_off": g["chunk_off"],
            "stage_cap": stage_cap,
            "agg_cap": agg_cap,
            "repeat": repeat,
            "fp8": fp8,
            "gsz": gsz,
            "mixed": mixed,
            "gbufs": gbufs,
            "s01sync": s01sync,
            "pobufs": pobufs,
        }
        _CACHE[cfg_key] = build_nc(cfg)
    nc = _CACHE[cfg_key]

    xp = np.zeros((NCORES * n_loc_pad, k_pad), np.float32)
    for c in range(NCORES):
        xp[c * n_loc_pad : c * n_loc_pad + n_loc, :f_in] = x[
            c * n_loc : (c + 1) * n_loc
        ]
    xpb = xp.astype(NPBF16)

    in_maps = []
    for c in range(NCORES):
        xT_loc = np.ascontiguousarray(
            xpb[c * n_loc_pad : (c + 1) * n_loc_pad].T
        )
        in_maps.append(
            {
                "xT": xT_loc,
                "W1": w["W1"],
                "W2": w["W2"],
                "Wgcn1": w["Wgcn1"],
                "Wgcn2": w["Wgcn2"],
                "Wfuse": w["Wfuse"],
                "Wc1": w["Wc1"],
                "Wc2": w["Wc2"],
                "Wc3": w["Wc3"],
                "cblob": w["cblob"],
                "cbT": w["cbT"],
                "ident": w["ident"],
                "idx16": g["idx16"][c],
                "s01": g["s01"][c],
                "s01T": g["s01T"][c],
                "sg": g["sg"][c],
            }
        )
    return nc, in_maps, n_loc, n_loc_pad


def kernel(**inputs):
    nc, in_maps, n_loc, n_loc_pad = prepare(inputs)
    res = run_bass_kernel_spmd(nc, in_maps, core_ids=list(range(NCORES)))
    n = np.asarray(inputs["x"]).shape[0]
    out = np.zeros((n, 5), np.float32)
    for c in range(NCORES):
        out[c * n_loc : (c + 1) * n_loc] = res.results[c]["out"][:5, :n_loc].T
    return out


if __name__ == "__main__":
    # quick smoke: tiny random problem shaped like the real one
    rng = np.random.default_rng(0)
    N, E, F_IN = 256, 2048, 96
    ip = {
        "x": rng.standard_normal((N, F_IN), dtype=np.float32),
        "edge_index": rng.integers(0, N, (2, E)),
        "W_gat1": rng.standard_normal((F_IN, 768), dtype=np.float32) * 0.05,
        "a_src1": rng.standard_normal((6, 128), dtype=np.float32) * 0.05,
        "a_dst1": rng.standard_normal((6, 128), dtype=np.float32) * 0.05,
        "b_gat1": np.zeros(128, np.float32),
        "W_gcn1": rng.standard_normal((128, 128), dtype=np.float32) * 0.05,
        "b_gcn1": np.zeros(128, np.float32),
        "W_gat2": rng.standard_normal((128, 512), dtype=np.float32) * 0.05,
        "a_src2": rng.standard_normal((4, 128), dtype=np.float32) * 0.05,
        "a_dst2": rng.standard_normal((4, 128), dtype=np.float32) * 0.05,
        "b_gat2": np.zeros(128, np.float32),
        "W_gcn2": rng.standard_normal((128, 64), dtype=np.float32) * 0.05,
        "b_gcn2": np.zeros(64, np.float32),
        "W_skip": rng.standard_normal((F_IN, 64), dtype=np.float32) * 0.05,
        "b_skip": np.zeros(64, np.float32),
        "W_fuse": rng.standard_normal((192, 64), dtype=np.float32) * 0.05,
        "b_fuse": np.zeros(64, np.float32),
        "W_c1": rng.standard_normal((64, 32), dtype=np.float32) * 0.05,
        "b_c1": np.zeros(32, np.float32),
        "W_c2": rng.standard_normal((32, 16), dtype=np.float32) * 0.05,
        "b_c2": np.zeros(16, np.float32),
        "W_c3": rng.standard_normal((16, 5), dtype=np.float32) * 0.05,
        "b_c3": np.zeros(5, np.float32),
        "g1": np.ones(128, np.float32), "be1": np.zeros(128, np.float32),
        "g2": np.ones(128, np.float32), "be2": np.zeros(128, np.float32),
        "g3": np.ones(64, np.float32), "be3": np.zeros(64, np.float32),
    }
    t0 = time.time()
    outv = kernel(**ip)
    print("kernel ran in", time.time() - t0, "shape", outv.shape)
    print(outv[:4])

